# revision 15
# baseline (speedup 1.0000x reference)
"""Trainium2 Bass kernel for nn_MiniDecoderBlock (B=2, T=2048, D=1024, H=16, DI=2048).

Strategy: 8-way tensor-parallel attention (2 heads/core, both batches),
one chunked ReduceScatter of the o_proj partial sums distributing tokens,
then token-sharded FFN (512 tokens/core, full d_inner).

kernel(**inputs) takes the FULL unsharded inputs and returns the FULL
output; sharding/compile/run happen inside.

Layout conventions (device side, per core):
  - Activations feature-major: xT [D, tokens] so matmul contraction (partition
    dim) is the feature dim.
  - Scores computed transposed: scoresT [k_tokens(P), q_tokens(free)] so the
    PV matmul uses stationary V and lands yT feature-major for o_proj.
  - V stored token-major with an appended ones column (sumexp for free).
  - rmsnorm applied via a gpsimd broadcast of the rms row onto all partitions,
    multiplied into q/k/v at the mandatory PSUM->SBUF copy.
  - ReduceScatter distributes attention partial sums by token blocks; core r
    owns global 128-token blocks {8c + r}.

Perf notes vs the first working version:
  - Activation tables: Ln/Exp/Square all forced into the combined
    natural_log_exp set (cache surgery on get_activation_tables) so the
    softmax stream never reloads the ACT table; FFN rsqrt runs on DVE
    (Quake-style seed + 2 Newton steps) so only silu ever switches sets.
  - Attention epilogue (recip/broadcast/normalize) is covered by reserved
    PE filler work instead of idling the PE.
  - Scores/PV operands in bf16; o_proj PSUM->SBUF copies on ACT.
  - Bulk DMAs batched with rearrange APs (one per chunk / weight tensor).
  - down_w resident in SBUF.
"""

import numpy as np

import concourse.bass as bass
import concourse.mybir as mybir
import concourse.tile as tile
from concourse import bacc
from concourse.masks import make_identity
from concourse.tile import TileContext

F32 = mybir.dt.float32
F32R = mybir.dt.float32r
I32 = mybir.dt.int32
BF16 = mybir.dt.bfloat16

N_CORES = 8
B, T, D = 2, 2048, 1024
H, HD = 16, 64
DI = 2048
HPC = H // N_CORES          # heads per core = 2
NTOK = B * T                # 4096
NCHUNK = NTOK // 512        # 8 x 512-token chunks
NBLK = NTOK // 128          # 32 x 128-token blocks
EPS = 1e-6
NEG = -1e30


def _patch_act_tables(arch):
    """Collapse Ln/Exp/Square onto the combined natural_log_exp table so the
    compiler never ping-pongs ACT table loads between Ln and Exp sets."""
    try:
        from concourse.hw_specs import get_activation_tables
        A = mybir.ActivationFunctionType
        tabs = get_activation_tables(arch)
        if "natural_log_exp_and_others" not in tabs:
            return
        for nm in tabs:
            if nm == "natural_log_exp_and_others":
                break
            tabs[nm].discard(A.Exp)
            tabs[nm].discard(A.Ln)
            tabs[nm].discard(A.Square)
    except Exception:
        pass


def build_nc(ffn_w_dtype=BF16, reps=1, no_collective=False):
    nc = bacc.Bacc("TRN2", target_bir_lowering=False, debug=False,
                   num_devices=1 if no_collective else N_CORES)
    _patch_act_tables(nc.m.arch)

    xT = nc.dram_tensor("xT", [D, NTOK], BF16, kind="ExternalInput")
    x_own = nc.dram_tensor("x_own", [512, D], F32, kind="ExternalInput")
    qkvT = nc.dram_tensor("qkvT", [D, 3 * HPC * HD], BF16, kind="ExternalInput")
    o_wT = nc.dram_tensor("o_wT", [HPC * HD, D], BF16, kind="ExternalInput")
    gT = nc.dram_tensor("gT", [D, DI], ffn_w_dtype, kind="ExternalInput")
    uT = nc.dram_tensor("uT", [D, DI], ffn_w_dtype, kind="ExternalInput")
    dT = nc.dram_tensor("dT", [DI, D], ffn_w_dtype, kind="ExternalInput")
    out = nc.dram_tensor("out", [512, D], F32, kind="ExternalOutput")

    with TileContext(nc) as tc:
        emit(nc, tc, xT, x_own, qkvT, o_wT, gT, uT, dT, out, reps=reps,
             no_collective=no_collective)
    nc.compile()
    return nc


def emit(nc, tc, xT, x_own, qkvT, o_wT, gT, uT, dT, out, reps=1, no_collective=False):
    EXP = mybir.ActivationFunctionType.Exp
    LN = mybir.ActivationFunctionType.Ln
    SQUARE = mybir.ActivationFunctionType.Square
    SILU = mybir.ActivationFunctionType.Silu
    COPY = mybir.ActivationFunctionType.Copy
    MUL = mybir.AluOpType.mult
    ADD = mybir.AluOpType.add

    from contextlib import ExitStack
    ctx = ExitStack()
    consts = ctx.enter_context(tc.tile_pool(name="consts", bufs=1))
    dram = ctx.enter_context(tc.tile_pool(name="dram", bufs=1, space="DRAM"))
    psum = ctx.enter_context(tc.tile_pool(name="psum", bufs=2, space="PSUM"))
    sb = ctx.enter_context(tc.tile_pool(name="sb", bufs=2))

    # ---- constants ----
    ident_bf = consts.tile([128, 128], BF16, tag="ident_bf")
    make_identity(nc, ident_bf[:, :])
    ones_c = consts.tile([128, 1], BF16, tag="ones_c")
    nc.vector.memset(ones_c[:, :], 1.0)
    eps_col = consts.tile([128, 1], F32, tag="eps_col")
    nc.vector.memset(eps_col[:, :], EPS)
    # causal mask addend: -BIG where k > q within a 128x128 diagonal block
    madd = consts.tile([128, 128], F32, tag="madd")
    nc.gpsimd.memset(madd[:, :], 0.0)
    nc.gpsimd.affine_select(
        out=madd[:, :], in_=madd[:, :],
        compare_op=mybir.AluOpType.is_ge, fill=NEG,
        base=0, pattern=[[1, 128]], channel_multiplier=-1,
    )

    # ---- persistent SBUF ----
    qkvT_sb = consts.tile([128, 8 * 384], BF16, tag="qkvT_sb")
    nc.sync.dma_start(
        out=qkvT_sb[:, :].rearrange("p (k c) -> p k c", k=8),
        in_=qkvT[:, :].rearrange("(k p) c -> p k c", k=8))
    o_wT_sb = consts.tile([128, D], BF16, tag="o_wT_sb")
    nc.sync.dma_start(out=o_wT_sb[:, :], in_=o_wT[:, :])

    kT_all = consts.tile([128, NTOK], BF16, tag="kT_all")
    v_aug = consts.tile([128, HPC * NBLK * 65], BF16, tag="v_aug")
    nc.vector.memset(v_aug[:, :], 1.0)

    # FFN weights resident, bf16 -- loaded during early chunks
    g_sb = consts.tile([128, 8 * DI], gT.dtype, tag="g_sb")
    u_sb = consts.tile([128, 8 * DI], uT.dtype, tag="u_sb")

    # ---- DRAM bounce ----
    rs_in = dram.tile([NTOK, D], BF16, tag="rs_in")
    rs_out = dram.tile([512, D], BF16, tag="rs_out")

    for _rep in range(reps):
        # ================= main loop over 512-token chunks =================
        def stats(i):
            """Load xT chunk i + rms broadcast tile (emitted ~1.5 chunks ahead)."""
            csl = slice(i * 512, (i + 1) * 512)
            xt = sb.tile([128, 8 * 512], BF16, tag="xt", name=f"xt_{i}")
            nc.sync.dma_start(
                out=xt[:, :].rearrange("p (k c) -> p k c", k=8),
                in_=xT[:, csl].rearrange("(k p) c -> p k c", k=8))
            ss = psum.tile([1, 512], F32, tag="proj", bufs=3, name=f"ss_{i}")
            for kk in range(8):
                sq = sb.tile([128, 512], BF16, tag="sq", bufs=2,
                             name=f"sq_{i}_{kk}")
                nc.vector.tensor_tensor(out=sq[:, :],
                                        in0=xt[:, kk * 512:(kk + 1) * 512],
                                        in1=xt[:, kk * 512:(kk + 1) * 512],
                                        op=MUL)
                nc.tensor.matmul(ss[:, :], ones_c[:, :], sq[:, :],
                                 start=(kk == 0), stop=(kk == 7))
            lt = sb.tile([1, 512], F32, tag="lt", bufs=1, name=f"lt_{i}")
            nc.scalar.activation(lt[:, :], ss[:, :], LN,
                                 bias=eps_col[0:1, :], scale=1.0 / D)
            rms_row = sb.tile([1, 512], BF16, tag="rms_row", name=f"rmsr_{i}")
            nc.scalar.activation(rms_row[:, :], lt[:, :], EXP, scale=-0.5)
            rms_b = sb.tile([128, 512], BF16, tag="rms_b", name=f"rmsb_{i}")
            nc.gpsimd.partition_broadcast(rms_b[:, :], rms_row[0:1, :])
            return xt, rms_b

        def qkv_steps(i, st):
            """Projection for chunk i as filler closures sprinkled into the
            previous chunk's attention g-loop (PE fills exp-wait gaps)."""
            csl = slice(i * 512, (i + 1) * 512)
            xt, rms_b = st
            state = {}
            steps = []

            def mk_proj(w, off, kk):
                def f():
                    if kk == 0:
                        state[w] = psum.tile([128, 512], F32, tag="proj",
                                             bufs=3, name=f"pj_{w}_{i}")
                    nc.tensor.matmul(
                        state[w][:, :],
                        qkvT_sb[:, kk * 384 + off:kk * 384 + off + 128],
                        xt[:, kk * 512:(kk + 1) * 512],
                        start=(kk == 0), stop=(kk == 7))
                return f

            for w, off in (("q", 0), ("k", 128), ("v", 256)):
                for kk in range(8):
                    steps.append(mk_proj(w, off, kk))

            def mk_qk_epi():
                def f():
                    q_sb = sb.tile([128, 512], BF16, tag="q_sb", name=f"q_{i}")
                    state["q_sb"] = q_sb
                    nc.vector.tensor_tensor(out=q_sb[:, :], in0=state["q"][:, :],
                                            in1=rms_b[:, :], op=MUL)
                    nc.vector.tensor_tensor(out=kT_all[:, csl],
                                            in0=state["k"][:, :],
                                            in1=rms_b[:, :], op=MUL)
                    v_sb = sb.tile([128, 512], BF16, tag="v_sb", name=f"v_{i}")
                    state["v_sb"] = v_sb
                    nc.vector.tensor_tensor(out=v_sb[:, :], in0=state["v"][:, :],
                                            in1=rms_b[:, :], op=MUL)
                return f

            steps.append(mk_qk_epi())

            def mk_vt(h, j):
                def f():
                    gb = i * 4 + j
                    v_sb = state["v_sb"]
                    vt = psum.tile([128, 64], BF16, tag="proj", bufs=3)
                    nc.tensor.transpose(vt[:, :],
                                        v_sb[h * 64:(h + 1) * 64,
                                             j * 128:(j + 1) * 128],
                                        ident_bf[h * 64:(h + 1) * 64,
                                                  h * 64:(h + 1) * 64])
                    slot = (h * NBLK + gb) * 65
                    nc.vector.tensor_copy(v_aug[:, slot:slot + 64], vt[:, :])
                return f

            for h in range(HPC):
                for j in range(4):
                    steps.append(mk_vt(h, j))
            return steps, state

        def attn_both(i, q_sb, y2_sb, fillers=(), reserve=0):
            """Scores+softmax+PV for chunk i, then the normalize epilogue.
            Keeps `reserve` fillers back to feed the PE during the epilogue's
            DVE/Pool dependency chain."""
            fillers = list(fillers)
            b, li = divmod(i, 4)
            nblk = li * 4 + 4
            avail = max(0, len(fillers) - reserve)
            per_g = max(1, -(-avail // max(1, nblk)))
            yT = [psum.tile([65, 512], F32, tag="yT", bufs=2, name=f"yT_{i}_{h}")
                  for h in range(2)]
            for g in range(nblk):
                gb = b * 16 + g
                q_off = max(0, g - li * 4) * 128
                w = 512 - q_off
                scs = []
                for h in range(2):
                    sc = psum.tile([128, 512], F32, tag="sc", bufs=3,
                                   name=f"sc{h}")
                    # lhsT at partitions h*64..h*64+64 -> distinct PE row
                    # groups; the two matmuls run concurrently in the array.
                    nc.tensor.matmul(
                        sc[:, 0:w],
                        kT_all[h * 64:(h + 1) * 64, gb * 128:(gb + 1) * 128],
                        q_sb[h * 64:(h + 1) * 64, q_off:512],
                        start=True, stop=True)
                    scs.append(sc)
                for h in range(2):
                    sc = scs[h]
                    if g >= li * 4:
                        nc.vector.tensor_tensor(out=sc[:, 0:128],
                                                in0=sc[:, 0:128],
                                                in1=madd[:, :], op=ADD)
                    pT = sb.tile([128, 512], BF16, tag="pT", bufs=3,
                                 name=f"pT{h}")
                    nc.scalar.activation(pT[:, 0:w], sc[:, 0:w], EXP)
                    slot = (h * NBLK + gb) * 65
                    nc.tensor.matmul(
                        yT[h][:, q_off:512],
                        v_aug[:, slot:slot + 65],
                        pT[:, 0:w],
                        start=(g == 0), stop=(g == nblk - 1))
                for _ in range(per_g):
                    if len(fillers) > reserve:
                        fillers.pop(0)()
            # normalize epilogue: reciprocal of the sumexp row (direct from
            # PSUM), broadcast, apply -- the reserved fillers keep PE fed.
            for h in range(2):
                rec = sb.tile([1, 512], F32, tag="rec")
                nc.vector.reciprocal(rec[:, :], yT[h][64:65, :])
                rb = sb.tile([64, 512], F32, tag=f"rb{h}", bufs=1)
                nc.gpsimd.partition_broadcast(rb[:, :], rec[0:1, :])
                nc.vector.tensor_tensor(out=y2_sb[h * 64:(h + 1) * 64, :],
                                        in0=yT[h][0:64, :], in1=rb[:, :],
                                        op=MUL)
            while fillers:
                fillers.pop(0)()

        def o_proj(i, y2_sb):
            osb = sb.tile([128, 4 * D], BF16, tag="osb", bufs=1)
            for j in range(4):
                for n in range(2):
                    op = psum.tile([128, 512], F32, tag="sc", bufs=3)
                    nc.tensor.matmul(op[:, :],
                                     y2_sb[:, j * 128:(j + 1) * 128],
                                     o_wT_sb[:, n * 512:(n + 1) * 512],
                                     start=True, stop=True)
                    nc.scalar.activation(
                        osb[:, j * D + n * 512:j * D + (n + 1) * 512],
                        op[:, :], COPY)
            r0 = i * 512
            nc.sync.dma_start(
                out=rs_in[r0:r0 + 512, :].rearrange("(k p) c -> p k c", k=4),
                in_=osb[:, :].rearrange("p (k c) -> p k c", k=4))

        # ================= FFN on own 512 tokens =================
        def dve_rsqrt(v, tag):
            """y = rsqrt(v) on DVE: Quake seed + 2 Newton steps. v: [128,1] f32."""
            y = sb.tile([128, 1], F32, tag=f"{tag}y", name=f"{tag}y")
            a = sb.tile([128, 1], F32, tag=f"{tag}a", name=f"{tag}a")
            # seed: y_i = 0x5f3759df - (v_i >> 1)  (via xor/add trick)
            nc.vector.tensor_scalar(
                out=a[:, :].bitcast(I32), in0=v[:, :].bitcast(I32),
                scalar1=1, scalar2=None,
                op0=mybir.AluOpType.arith_shift_right)
            nc.vector.tensor_scalar(
                out=y[:, :].bitcast(I32), in0=a[:, :].bitcast(I32),
                scalar1=-1, scalar2=None,
                op0=mybir.AluOpType.bitwise_xor)
            nc.vector.tensor_scalar(
                out=y[:, :].bitcast(I32), in0=y[:, :].bitcast(I32),
                scalar1=0x5f3759e0, scalar2=None, op0=ADD)
            for _ in range(2):
                nc.vector.tensor_tensor(out=a[:, :], in0=v[:, :], in1=y[:, :],
                                        op=MUL)
                nc.vector.tensor_tensor(out=a[:, :], in0=a[:, :], in1=y[:, :],
                                        op=MUL)
                nc.vector.tensor_scalar(
                    out=a[:, :], in0=a[:, :], scalar1=-0.5, scalar2=1.5,
                    op0=MUL, op1=ADD)
                nc.vector.tensor_tensor(out=y[:, :], in0=y[:, :], in1=a[:, :],
                                        op=MUL)
            return y

        def ffn_prep(ha):
            x2t, xn2T, xn2s = [], [], []
            for jj in range(2):
                c2 = ha * 2 + jj
                rsx = sb.tile([128, D], BF16, tag="rsx", bufs=1, name=f"rsx{c2}")
                nc.sync.dma_start(out=rsx[:, :],
                                  in_=rs_out[c2 * 128:(c2 + 1) * 128, :])
                xo = sb.tile([128, D], F32, tag="xo", bufs=1, name=f"xo{c2}")
                nc.sync.dma_start(out=xo[:, :],
                                  in_=x_own[c2 * 128:(c2 + 1) * 128, :])
                x2 = sb.tile([128, D], BF16, tag=f"x2_{jj}", bufs=2,
                             name=f"x2_{c2}")
                nc.vector.tensor_tensor(out=x2[:, :], in0=rsx[:, :],
                                        in1=xo[:, :], op=ADD)
                x2t.append(x2)
                scr = sb.tile([128, D], BF16, tag="scr", bufs=1, name=f"scr{c2}")
                ss2 = sb.tile([128, 1], F32, tag="ss2", name=f"ss2_{c2}")
                nc.scalar.activation(scr[:, :], x2[:, :], SQUARE,
                                     accum_out=ss2[:, :])
                t2 = sb.tile([128, 1], F32, tag="t2", name=f"t2_{c2}")
                nc.vector.tensor_scalar(
                    out=t2[:, :], in0=ss2[:, :], scalar1=1.0 / D, scalar2=EPS,
                    op0=MUL, op1=ADD)
                r2 = dve_rsqrt(t2, tag=f"r2_{c2}")
                xn2 = sb.tile([128, D], BF16, tag=f"xn2_{jj}", bufs=2,
                              name=f"xn2_{c2}")
                nc.vector.tensor_scalar_mul(xn2[:, :], x2[:, :], r2[:, :])
                xn2s.append(xn2)
            for kk in range(8):
                xt2 = sb.tile([128, 256], gT.dtype, tag=f"xn2T{kk}",
                              bufs=2, name=f"xn2T{kk}_{ha}")
                xn2T.append(xt2)

            def mk_tp(jj, kk):
                def f():
                    xp = psum.tile([128, 128], BF16, tag="proj", bufs=3)
                    nc.tensor.transpose(xp[:, :],
                                        xn2s[jj][:, kk * 128:(kk + 1) * 128],
                                        ident_bf[:, :])
                    nc.vector.tensor_copy(xn2T[kk][:, jj * 128:(jj + 1) * 128],
                                          xp[:, :])
                return f

            tps = [mk_tp(jj, kk) for jj in range(2) for kk in range(8)]
            return x2t, xn2T, tps

        def ffn_mats(ha, x2t, xn2T, fillers=()):
            fillers = list(fillers)
            h_sb = []
            for m in range(16):
                if m >= 8 and fillers:
                    fillers.pop(0)()
                    if fillers:
                        fillers.pop(0)()
                gp = psum.tile([128, 256], F32, tag="sc", bufs=3)
                up = psum.tile([128, 256], F32, tag="sc", bufs=3)
                for kk in range(8):
                    nc.tensor.matmul(gp[:, :],
                                     g_sb[:, kk * DI + m * 128:kk * DI + (m + 1) * 128],
                                     xn2T[kk][:, :],
                                     start=(kk == 0), stop=(kk == 7))
                for kk in range(8):
                    nc.tensor.matmul(up[:, :],
                                     u_sb[:, kk * DI + m * 128:kk * DI + (m + 1) * 128],
                                     xn2T[kk][:, :],
                                     start=(kk == 0), stop=(kk == 7))
                sg = sb.tile([128, 256], BF16, tag="sg")
                nc.scalar.activation(sg[:, :], gp[:, :], SILU)
                hm = sb.tile([128, 256], dT.dtype, tag=f"h{m}", bufs=1,
                             name=f"h{m}_{ha}")
                nc.vector.tensor_tensor(out=hm[:, :], in0=sg[:, :],
                                        in1=up[:, :], op=MUL)
                h_sb.append(hm)

            while fillers:
                fillers.pop(0)()
            dp = [psum.tile([128, 512], F32,
                            tag="proj" if nn == 0 else "sc", bufs=3,
                            name=f"dp{jj}_{nn}")
                  for nn in range(2) for jj in range(2)]
            # down_w streamed in 4 groups of 4 m-blocks, double buffered
            dts = {}

            def dt_load(grp):
                dt = sb.tile([128, 4 * D], dT.dtype, tag="dt4", bufs=2)
                nc.sync.dma_start(
                    out=dt[:, :].rearrange("p (k c) -> p k c", k=4),
                    in_=dT[grp * 512:(grp + 1) * 512, :].rearrange(
                        "(k p) c -> p k c", k=4))
                dts[grp] = dt

            dt_load(0)
            dt_load(1)
            for m in range(16):
                grp, mm = divmod(m, 4)
                if mm == 0 and grp + 2 < 4:
                    dt_load(grp + 2)
                for n in range(2):
                    for jj in range(2):
                        nc.tensor.matmul(
                            dp[n * 2 + jj][:, :],
                            h_sb[m][:, jj * 128:(jj + 1) * 128],
                            dts[grp][:, mm * D + n * 512:mm * D + (n + 1) * 512],
                            start=(m == 0), stop=(m == 15))
            for jj in range(2):
                c2 = ha * 2 + jj
                osb = sb.tile([128, D], F32, tag="fout", bufs=2)
                for n in range(2):
                    nc.vector.tensor_tensor(out=osb[:, n * 512:(n + 1) * 512],
                                            in0=dp[n * 2 + jj][:, :],
                                            in1=x2t[jj][:, n * 512:(n + 1) * 512],
                                            op=ADD)
                nc.sync.dma_start(out=out[c2 * 128:(c2 + 1) * 128, :],
                                  in_=osb[:, :])


        st = stats(0)
        steps0, state0 = qkv_steps(0, st)
        for f in steps0:
            f()
        q_cur = state0["q_sb"]
        st_next = stats(1)
        state_next = None
        ffn0 = None
        for i in range(NCHUNK):
            y2_sb = sb.tile([128, 512], BF16, tag="y2_sb", name=f"y2_{i}")
            if i + 1 < NCHUNK:
                fillers, state_next = qkv_steps(i + 1, st_next)
            else:
                # last chunk: feed the g-loop + epilogue with the first FFN
                # half's transposes instead of qkv work
                x2t0, xn2T0, tps0 = ffn0
                fillers = list(tps0)
            attn_both(i, q_cur, y2_sb, fillers, reserve=12 if fillers else 0)
            if i == 1:
                nc.sync.dma_start(
                    out=g_sb[:, :].rearrange("p (k c) -> p k c", k=8),
                    in_=gT[:, :].rearrange("(k p) c -> p k c", k=8))
                nc.sync.dma_start(
                    out=u_sb[:, :].rearrange("p (k c) -> p k c", k=8),
                    in_=uT[:, :].rearrange("(k p) c -> p k c", k=8))

            if i + 2 < NCHUNK:
                st_next = stats(i + 2)
            o_proj(i, y2_sb)
            if i + 1 < NCHUNK:
                q_cur = state_next["q_sb"]
            if i == 5:
                ffn0 = ffn_prep(0)
            if i % 2 == 1:
                c = i // 2
                if no_collective:
                    nc.sync.dma_start(
                        out=rs_out[c * 128:(c + 1) * 128, :],
                        in_=rs_in[c * 1024:c * 1024 + 128, :])
                else:
                    nc.gpsimd.collective_compute(
                        "ReduceScatter", mybir.AluOpType.add,
                        ins=[rs_in[c * 1024:(c + 1) * 1024, :]],
                        outs=[rs_out[c * 128:(c + 1) * 128, :]],
                        replica_groups=[list(range(N_CORES))],
                    )

        x2t1, xn2T1, tps1 = ffn_prep(1)
        ffn_mats(0, x2t0, xn2T0, tps1)
        ffn_mats(1, x2t1, xn2T1)

    ctx.close()


# ===================== host-side sharding =====================

def make_in_maps(x, ln1_w, ln2_w, qkv_w, o_w, gate_w, up_w, down_w,
                 ffn_np_dtype=None):
    import ml_dtypes
    if ffn_np_dtype is None:
        ffn_np_dtype = ml_dtypes.bfloat16
    x = np.asarray(x, np.float32)
    xf = np.ascontiguousarray(x.reshape(NTOK, D))
    xT = np.ascontiguousarray(xf.T).astype(ml_dtypes.bfloat16)

    qkv_eff = np.asarray(qkv_w, np.float32) * np.asarray(ln1_w, np.float32)[None, :]
    g_eff = np.asarray(gate_w, np.float32) * np.asarray(ln2_w, np.float32)[None, :]
    u_eff = np.asarray(up_w, np.float32) * np.asarray(ln2_w, np.float32)[None, :]
    o_w = np.asarray(o_w, np.float32)
    down_w = np.asarray(down_w, np.float32)

    gT = np.ascontiguousarray(g_eff.T).astype(ffn_np_dtype)
    uT = np.ascontiguousarray(u_eff.T).astype(ffn_np_dtype)
    dT = np.ascontiguousarray(down_w.T).astype(ffn_np_dtype)

    scale = 1.0 / np.sqrt(HD)
    in_maps = []
    for r in range(N_CORES):
        hsl = slice(r * HPC * HD, (r + 1) * HPC * HD)  # rows for this core's heads
        qr = qkv_eff[hsl, :] * scale          # [128, D] pre-scaled q
        kr = qkv_eff[D + r * 128:D + (r + 1) * 128, :]
        vr = qkv_eff[2 * D + r * 128:2 * D + (r + 1) * 128, :]
        qkvT_r = np.ascontiguousarray(
            np.concatenate([qr, kr, vr], axis=0).T).astype(ml_dtypes.bfloat16)
        o_wT_r = np.ascontiguousarray(o_w[:, hsl].T).astype(ml_dtypes.bfloat16)
        xo = np.ascontiguousarray(
            xf.reshape(NBLK, 128, D)[r::N_CORES].reshape(512, D))
        in_maps.append({
            "xT": xT, "x_own": xo, "qkvT": qkvT_r, "o_wT": o_wT_r,
            "gT": gT, "uT": uT, "dT": dT,
        })
    return in_maps


def assemble_out(results):
    outf = np.empty((NTOK, D), np.float32)
    for r in range(N_CORES):
        outf.reshape(NBLK, 128, D)[r::N_CORES] = \
            results[r]["out"].reshape(4, 128, D)
    return outf.reshape(B, T, D)


# ===================== entry point =====================

_NC_CACHE = {}


def _get_nc():
    if "nc" not in _NC_CACHE:
        _NC_CACHE["nc"] = build_nc()
    return _NC_CACHE["nc"]


def kernel(x, ln1_w, ln2_w, qkv_w, o_w, gate_w, up_w, down_w):
    from concourse.bass_utils import run_bass_kernel_spmd

    nc = _get_nc()
    in_maps = make_in_maps(x, ln1_w, ln2_w, qkv_w, o_w, gate_w, up_w, down_w)
    res = run_bass_kernel_spmd(nc, in_maps, core_ids=list(range(N_CORES)))
    return assemble_out(res.results)


# revision 20
# speedup vs baseline: 1.9088x; 1.9088x over previous
"""Trainium2 Bass kernel for nn_MiniDecoderBlock (B=2, T=2048, D=1024, H=16, DI=2048).

Strategy: 8-way tensor-parallel attention (2 heads/core, both batches),
one chunked ReduceScatter of the o_proj partial sums distributing tokens,
then token-sharded FFN (512 tokens/core, full d_inner).

kernel(**inputs) takes the FULL unsharded inputs and returns the FULL
output; sharding/compile/run happen inside.

Layout conventions (device side, per core):
  - Activations feature-major: xT [D, tokens] so matmul contraction (partition
    dim) is the feature dim.
  - Scores computed transposed: scoresT [k_tokens(P), q_tokens(free)] so the
    PV matmul uses stationary V and lands yT feature-major for o_proj.
  - V stored token-major with an appended ones column (sumexp for free).
  - rmsnorm applied via a gpsimd broadcast of the rms row onto all partitions,
    multiplied into q/k/v at the mandatory PSUM->SBUF copy.
  - ReduceScatter distributes attention partial sums by token blocks; core r
    owns global 128-token blocks {8c + r}.

Perf notes vs the first working version:
  - Activation tables: Ln/Exp/Square all forced into the combined
    natural_log_exp set (cache surgery on get_activation_tables) so the
    softmax stream never reloads the ACT table; FFN rsqrt runs on DVE
    (Quake-style seed + 2 Newton steps) so only silu ever switches sets.
  - Attention epilogue (recip/broadcast/normalize) is covered by reserved
    PE filler work instead of idling the PE.
  - Scores/PV operands in bf16; o_proj PSUM->SBUF copies on ACT.
  - Bulk DMAs batched with rearrange APs (one per chunk / weight tensor).
  - down_w resident in SBUF.
"""

import numpy as np

import concourse.bass as bass
import concourse.mybir as mybir
import concourse.tile as tile
from concourse import bacc
from concourse.masks import make_identity
from concourse.tile import TileContext

F32 = mybir.dt.float32
F32R = mybir.dt.float32r
I32 = mybir.dt.int32
BF16 = mybir.dt.bfloat16

N_CORES = 8
B, T, D = 2, 2048, 1024
H, HD = 16, 64
DI = 2048
HPC = H // N_CORES          # heads per core = 2
NTOK = B * T                # 4096
NCHUNK = NTOK // 512        # 8 x 512-token chunks
NBLK = NTOK // 128          # 32 x 128-token blocks
EPS = 1e-6
NEG = -1e30


def _patch_act_tables(arch):
    """Collapse Ln/Exp/Square onto the combined natural_log_exp table so the
    compiler never ping-pongs ACT table loads between Ln and Exp sets."""
    try:
        from concourse.hw_specs import get_activation_tables
        A = mybir.ActivationFunctionType
        tabs = get_activation_tables(arch)
        if "natural_log_exp_and_others" not in tabs:
            return
        for nm in tabs:
            if nm == "natural_log_exp_and_others":
                break
            tabs[nm].discard(A.Exp)
            tabs[nm].discard(A.Ln)
            tabs[nm].discard(A.Square)
    except Exception:
        pass


def build_nc(ffn_w_dtype=BF16, reps=1, no_collective=False):
    nc = bacc.Bacc("TRN2", target_bir_lowering=False, debug=False,
                   num_devices=1 if no_collective else N_CORES)
    _patch_act_tables(nc.m.arch)

    xT = nc.dram_tensor("xT", [D, NTOK], BF16, kind="ExternalInput")
    x_own = nc.dram_tensor("x_own", [512, D], F32, kind="ExternalInput")
    qkvT = nc.dram_tensor("qkvT", [D, 3 * HPC * HD], BF16, kind="ExternalInput")
    o_wT = nc.dram_tensor("o_wT", [HPC * HD, D], BF16, kind="ExternalInput")
    gT = nc.dram_tensor("gT", [D, DI], ffn_w_dtype, kind="ExternalInput")
    uT = nc.dram_tensor("uT", [D, DI], ffn_w_dtype, kind="ExternalInput")
    dT = nc.dram_tensor("dT", [DI, D], ffn_w_dtype, kind="ExternalInput")
    out = nc.dram_tensor("out", [512, D], F32, kind="ExternalOutput")

    with TileContext(nc) as tc:
        emit(nc, tc, xT, x_own, qkvT, o_wT, gT, uT, dT, out, reps=reps,
             no_collective=no_collective)
    nc.compile()
    return nc


def emit(nc, tc, xT, x_own, qkvT, o_wT, gT, uT, dT, out, reps=1, no_collective=False):
    EXP = mybir.ActivationFunctionType.Exp
    LN = mybir.ActivationFunctionType.Ln
    SQUARE = mybir.ActivationFunctionType.Square
    SILU = mybir.ActivationFunctionType.Silu
    COPY = mybir.ActivationFunctionType.Copy
    MUL = mybir.AluOpType.mult
    ADD = mybir.AluOpType.add

    from contextlib import ExitStack
    ctx = ExitStack()
    consts = ctx.enter_context(tc.tile_pool(name="consts", bufs=1))
    dram = ctx.enter_context(tc.tile_pool(name="dram", bufs=1, space="DRAM"))
    psum = ctx.enter_context(tc.tile_pool(name="psum", bufs=2, space="PSUM"))
    sb = ctx.enter_context(tc.tile_pool(name="sb", bufs=2))

    # ---- constants ----
    ident_bf = consts.tile([128, 128], BF16, tag="ident_bf")
    make_identity(nc, ident_bf[:, :])
    ones_c = consts.tile([128, 1], BF16, tag="ones_c")
    nc.vector.memset(ones_c[:, :], 1.0)
    eps_col = consts.tile([128, 1], F32, tag="eps_col")
    nc.vector.memset(eps_col[:, :], EPS)
    # causal mask addend: -BIG where k > q within a 128x128 diagonal block
    madd = consts.tile([128, 128], F32, tag="madd")
    nc.gpsimd.memset(madd[:, :], 0.0)
    nc.gpsimd.affine_select(
        out=madd[:, :], in_=madd[:, :],
        compare_op=mybir.AluOpType.is_ge, fill=NEG,
        base=0, pattern=[[1, 128]], channel_multiplier=-1,
    )

    # ---- persistent SBUF ----
    qkvT_sb = consts.tile([128, 8 * 384], BF16, tag="qkvT_sb")
    for kk in range(8):
        nc.sync.dma_start(out=qkvT_sb[:, kk * 384:(kk + 1) * 384],
                          in_=qkvT[kk * 128:(kk + 1) * 128, :])
    o_wT_sb = consts.tile([128, D], BF16, tag="o_wT_sb")
    nc.sync.dma_start(out=o_wT_sb[:, :], in_=o_wT[:, :])

    kT_all = consts.tile([128, NTOK], BF16, tag="kT_all")
    v_aug = consts.tile([128, HPC * NBLK * 65], BF16, tag="v_aug")
    nc.vector.memset(v_aug[:, :], 1.0)

    # FFN weights resident, bf16 -- loaded during early chunks
    g_sb = consts.tile([128, 8 * DI], gT.dtype, tag="g_sb")
    u_sb = consts.tile([128, 8 * DI], uT.dtype, tag="u_sb")

    # ---- DRAM bounce ----
    rs_in = dram.tile([NTOK, D], BF16, tag="rs_in")
    rs_out = dram.tile([512, D], BF16, tag="rs_out")

    for _rep in range(reps):
        # ================= main loop over 512-token chunks =================
        def stats(i):
            """Load xT chunk i + rms broadcast tile (emitted ~1.5 chunks ahead)."""
            csl = slice(i * 512, (i + 1) * 512)
            xt = sb.tile([128, 8 * 512], BF16, tag="xt", name=f"xt_{i}")
            for kk in range(8):
                nc.sync.dma_start(out=xt[:, kk * 512:(kk + 1) * 512],
                                  in_=xT[kk * 128:(kk + 1) * 128, csl])
            ss = psum.tile([1, 512], F32, tag="proj", bufs=3, name=f"ss_{i}")
            for kk in range(8):
                sq = sb.tile([128, 512], BF16, tag="sq", bufs=2,
                             name=f"sq_{i}_{kk}")
                nc.vector.tensor_tensor(out=sq[:, :],
                                        in0=xt[:, kk * 512:(kk + 1) * 512],
                                        in1=xt[:, kk * 512:(kk + 1) * 512],
                                        op=MUL)
                nc.tensor.matmul(ss[:, :], ones_c[:, :], sq[:, :],
                                 start=(kk == 0), stop=(kk == 7))
            lt = sb.tile([1, 512], F32, tag="lt", bufs=1, name=f"lt_{i}")
            nc.scalar.activation(lt[:, :], ss[:, :], LN,
                                 bias=eps_col[0:1, :], scale=1.0 / D)
            rms_row = sb.tile([1, 512], BF16, tag="rms_row", name=f"rmsr_{i}")
            nc.scalar.activation(rms_row[:, :], lt[:, :], EXP, scale=-0.5)
            rms_b = sb.tile([128, 512], BF16, tag="rms_b", name=f"rmsb_{i}")
            nc.gpsimd.partition_broadcast(rms_b[:, :], rms_row[0:1, :])
            return xt, rms_b

        def qkv_steps(i, st):
            """Projection for chunk i as filler closures sprinkled into the
            previous chunk's attention g-loop (PE fills exp-wait gaps)."""
            csl = slice(i * 512, (i + 1) * 512)
            xt, rms_b = st
            state = {}
            steps = []

            def mk_proj(w, off, kk):
                def f():
                    if kk == 0:
                        state[w] = psum.tile([128, 512], F32, tag="proj",
                                             bufs=3, name=f"pj_{w}_{i}")
                    nc.tensor.matmul(
                        state[w][:, :],
                        qkvT_sb[:, kk * 384 + off:kk * 384 + off + 128],
                        xt[:, kk * 512:(kk + 1) * 512],
                        start=(kk == 0), stop=(kk == 7))
                return f

            for w, off in (("q", 0), ("k", 128), ("v", 256)):
                for kk in range(8):
                    steps.append(mk_proj(w, off, kk))

            def mk_qk_epi():
                def f():
                    q_sb = sb.tile([128, 512], BF16, tag="q_sb", name=f"q_{i}")
                    state["q_sb"] = q_sb
                    nc.vector.tensor_tensor(out=q_sb[:, :], in0=state["q"][:, :],
                                            in1=rms_b[:, :], op=MUL)
                    nc.vector.tensor_tensor(out=kT_all[:, csl],
                                            in0=state["k"][:, :],
                                            in1=rms_b[:, :], op=MUL)
                    v_sb = sb.tile([128, 512], BF16, tag="v_sb", name=f"v_{i}")
                    state["v_sb"] = v_sb
                    nc.vector.tensor_tensor(out=v_sb[:, :], in0=state["v"][:, :],
                                            in1=rms_b[:, :], op=MUL)
                return f

            steps.append(mk_qk_epi())

            def mk_vt(h, j):
                def f():
                    gb = i * 4 + j
                    v_sb = state["v_sb"]
                    vt = psum.tile([128, 64], BF16, tag="proj", bufs=3)
                    nc.tensor.transpose(vt[:, :],
                                        v_sb[h * 64:(h + 1) * 64,
                                             j * 128:(j + 1) * 128],
                                        ident_bf[h * 64:(h + 1) * 64,
                                                  h * 64:(h + 1) * 64])
                    slot = (h * NBLK + gb) * 65
                    nc.vector.tensor_copy(v_aug[:, slot:slot + 64], vt[:, :])
                return f

            for h in range(HPC):
                for j in range(4):
                    steps.append(mk_vt(h, j))
            return steps, state

        def attn_both(i, q_sb, y2_sb, fillers=(), reserve=0):
            """Scores+softmax+PV for chunk i, then the normalize epilogue.
            Keeps `reserve` fillers back to feed the PE during the epilogue's
            DVE/Pool dependency chain."""
            fillers = list(fillers)
            b, li = divmod(i, 4)
            nblk = li * 4 + 4
            avail = max(0, len(fillers) - reserve)
            per_g = max(1, -(-avail // max(1, nblk)))
            yT = [psum.tile([65, 512], F32, tag="yT", bufs=2, name=f"yT_{i}_{h}")
                  for h in range(2)]
            for g in range(nblk):
                gb = b * 16 + g
                q_off = max(0, g - li * 4) * 128
                w = 512 - q_off
                scs = []
                for h in range(2):
                    sc = psum.tile([128, 512], F32, tag="sc", bufs=3,
                                   name=f"sc{h}")
                    # lhsT at partitions h*64..h*64+64 -> distinct PE row
                    # groups; the two matmuls run concurrently in the array.
                    nc.tensor.matmul(
                        sc[:, 0:w],
                        kT_all[h * 64:(h + 1) * 64, gb * 128:(gb + 1) * 128],
                        q_sb[h * 64:(h + 1) * 64, q_off:512],
                        start=True, stop=True)
                    scs.append(sc)
                for h in range(2):
                    sc = scs[h]
                    if g >= li * 4:
                        nc.vector.tensor_tensor(out=sc[:, 0:128],
                                                in0=sc[:, 0:128],
                                                in1=madd[:, :], op=ADD)
                    pT = sb.tile([128, 512], BF16, tag="pT", bufs=3,
                                 name=f"pT{h}")
                    nc.scalar.activation(pT[:, 0:w], sc[:, 0:w], EXP)
                    slot = (h * NBLK + gb) * 65
                    nc.tensor.matmul(
                        yT[h][:, q_off:512],
                        v_aug[:, slot:slot + 65],
                        pT[:, 0:w],
                        start=(g == 0), stop=(g == nblk - 1))
                for _ in range(per_g):
                    if len(fillers) > reserve:
                        fillers.pop(0)()
            # normalize epilogue: reciprocal of the sumexp row (direct from
            # PSUM), broadcast, apply -- the reserved fillers keep PE fed.
            for h in range(2):
                rec = sb.tile([1, 512], F32, tag="rec")
                nc.vector.reciprocal(rec[:, :], yT[h][64:65, :])
                rb = sb.tile([64, 512], F32, tag=f"rb{h}", bufs=1)
                nc.gpsimd.partition_broadcast(rb[:, :], rec[0:1, :])
                nc.vector.tensor_tensor(out=y2_sb[h * 64:(h + 1) * 64, :],
                                        in0=yT[h][0:64, :], in1=rb[:, :],
                                        op=MUL)
            while fillers:
                fillers.pop(0)()

        def o_proj(i, y2_sb):
            osb = sb.tile([128, 4 * D], BF16, tag="osb", bufs=1)
            for j in range(4):
                for n in range(2):
                    op = psum.tile([128, 512], F32, tag="sc", bufs=3)
                    nc.tensor.matmul(op[:, :],
                                     y2_sb[:, j * 128:(j + 1) * 128],
                                     o_wT_sb[:, n * 512:(n + 1) * 512],
                                     start=True, stop=True)
                    nc.scalar.activation(
                        osb[:, j * D + n * 512:j * D + (n + 1) * 512],
                        op[:, :], COPY)
            for j in range(4):
                r0 = i * 512 + j * 128
                nc.sync.dma_start(out=rs_in[r0:r0 + 128, :],
                                  in_=osb[:, j * D:(j + 1) * D])

        # ================= FFN on own 512 tokens =================
        def dve_rsqrt(v, tag):
            """y = rsqrt(v) on DVE: Quake seed + 2 Newton steps. v: [128,1] f32."""
            y = sb.tile([128, 1], F32, tag=f"{tag}y", name=f"{tag}y")
            a = sb.tile([128, 1], F32, tag=f"{tag}a", name=f"{tag}a")
            # seed: y_i = 0x5f3759df - (v_i >> 1)  (via xor/add trick)
            nc.vector.tensor_scalar(
                out=a[:, :].bitcast(I32), in0=v[:, :].bitcast(I32),
                scalar1=1, scalar2=None,
                op0=mybir.AluOpType.arith_shift_right)
            nc.vector.tensor_scalar(
                out=y[:, :].bitcast(I32), in0=a[:, :].bitcast(I32),
                scalar1=-1, scalar2=None,
                op0=mybir.AluOpType.bitwise_xor)
            nc.vector.tensor_scalar(
                out=y[:, :].bitcast(I32), in0=y[:, :].bitcast(I32),
                scalar1=0x5f3759e0, scalar2=None, op0=ADD)
            for _ in range(2):
                nc.vector.tensor_tensor(out=a[:, :], in0=v[:, :], in1=y[:, :],
                                        op=MUL)
                nc.vector.tensor_tensor(out=a[:, :], in0=a[:, :], in1=y[:, :],
                                        op=MUL)
                nc.vector.tensor_scalar(
                    out=a[:, :], in0=a[:, :], scalar1=-0.5, scalar2=1.5,
                    op0=MUL, op1=ADD)
                nc.vector.tensor_tensor(out=y[:, :], in0=y[:, :], in1=a[:, :],
                                        op=MUL)
            return y

        def ffn_prep(ha):
            x2t, xn2T, xn2s = [], [], []
            for jj in range(2):
                c2 = ha * 2 + jj
                rsx = sb.tile([128, D], BF16, tag="rsx", bufs=1, name=f"rsx{c2}")
                nc.sync.dma_start(out=rsx[:, :],
                                  in_=rs_out[c2 * 128:(c2 + 1) * 128, :])
                xo = sb.tile([128, D], F32, tag="xo", bufs=1, name=f"xo{c2}")
                nc.sync.dma_start(out=xo[:, :],
                                  in_=x_own[c2 * 128:(c2 + 1) * 128, :])
                x2 = sb.tile([128, D], BF16, tag=f"x2_{jj}", bufs=2,
                             name=f"x2_{c2}")
                nc.vector.tensor_tensor(out=x2[:, :], in0=rsx[:, :],
                                        in1=xo[:, :], op=ADD)
                x2t.append(x2)
                scr = sb.tile([128, D], BF16, tag="scr", bufs=1, name=f"scr{c2}")
                ss2 = sb.tile([128, 1], F32, tag="ss2", name=f"ss2_{c2}")
                nc.scalar.activation(scr[:, :], x2[:, :], SQUARE,
                                     accum_out=ss2[:, :])
                t2 = sb.tile([128, 1], F32, tag="t2", name=f"t2_{c2}")
                nc.vector.tensor_scalar(
                    out=t2[:, :], in0=ss2[:, :], scalar1=1.0 / D, scalar2=EPS,
                    op0=MUL, op1=ADD)
                r2 = dve_rsqrt(t2, tag=f"r2_{c2}")
                xn2 = sb.tile([128, D], BF16, tag=f"xn2_{jj}", bufs=2,
                              name=f"xn2_{c2}")
                nc.vector.tensor_scalar_mul(xn2[:, :], x2[:, :], r2[:, :])
                xn2s.append(xn2)
            for kk in range(8):
                xt2 = sb.tile([128, 256], gT.dtype, tag=f"xn2T{kk}",
                              bufs=2, name=f"xn2T{kk}_{ha}")
                xn2T.append(xt2)

            def mk_tp(jj, kk):
                def f():
                    xp = psum.tile([128, 128], BF16, tag="proj", bufs=3)
                    nc.tensor.transpose(xp[:, :],
                                        xn2s[jj][:, kk * 128:(kk + 1) * 128],
                                        ident_bf[:, :])
                    nc.vector.tensor_copy(xn2T[kk][:, jj * 128:(jj + 1) * 128],
                                          xp[:, :])
                return f

            tps = [mk_tp(jj, kk) for jj in range(2) for kk in range(8)]
            return x2t, xn2T, tps

        def ffn_mats(ha, x2t, xn2T, fillers=()):
            fillers = list(fillers)
            h_sb = []
            for m in range(16):
                if m >= 8 and fillers:
                    fillers.pop(0)()
                    if fillers:
                        fillers.pop(0)()
                gp = psum.tile([128, 256], F32, tag="sc", bufs=3)
                up = psum.tile([128, 256], F32, tag="sc", bufs=3)
                for kk in range(8):
                    nc.tensor.matmul(gp[:, :],
                                     g_sb[:, kk * DI + m * 128:kk * DI + (m + 1) * 128],
                                     xn2T[kk][:, :],
                                     start=(kk == 0), stop=(kk == 7))
                for kk in range(8):
                    nc.tensor.matmul(up[:, :],
                                     u_sb[:, kk * DI + m * 128:kk * DI + (m + 1) * 128],
                                     xn2T[kk][:, :],
                                     start=(kk == 0), stop=(kk == 7))
                sg = sb.tile([128, 256], BF16, tag="sg")
                nc.scalar.activation(sg[:, :], gp[:, :], SILU)
                hm = sb.tile([128, 256], dT.dtype, tag=f"h{m}", bufs=1,
                             name=f"h{m}_{ha}")
                nc.vector.tensor_tensor(out=hm[:, :], in0=sg[:, :],
                                        in1=up[:, :], op=MUL)
                h_sb.append(hm)

            while fillers:
                fillers.pop(0)()
            dp = [psum.tile([128, 512], F32,
                            tag="proj" if nn == 0 else "sc", bufs=3,
                            name=f"dp{jj}_{nn}")
                  for nn in range(2) for jj in range(2)]
            for m in range(16):
                dt = sb.tile([128, D], dT.dtype, tag="dt", bufs=4)
                nc.sync.dma_start(out=dt[:, :],
                                  in_=dT[m * 128:(m + 1) * 128, :])
                for n in range(2):
                    for jj in range(2):
                        nc.tensor.matmul(dp[n * 2 + jj][:, :],
                                         h_sb[m][:, jj * 128:(jj + 1) * 128],
                                         dt[:, n * 512:(n + 1) * 512],
                                         start=(m == 0), stop=(m == 15))
            for jj in range(2):
                c2 = ha * 2 + jj
                osb = sb.tile([128, D], F32, tag="fout", bufs=2)
                for n in range(2):
                    nc.vector.tensor_tensor(out=osb[:, n * 512:(n + 1) * 512],
                                            in0=dp[n * 2 + jj][:, :],
                                            in1=x2t[jj][:, n * 512:(n + 1) * 512],
                                            op=ADD)
                nc.sync.dma_start(out=out[c2 * 128:(c2 + 1) * 128, :],
                                  in_=osb[:, :])


        st = stats(0)
        steps0, state0 = qkv_steps(0, st)
        for f in steps0:
            f()
        q_cur = state0["q_sb"]
        st_next = stats(1)
        state_next = None
        ffn0 = None
        for i in range(NCHUNK):
            y2_sb = sb.tile([128, 512], BF16, tag="y2_sb", name=f"y2_{i}")
            if i + 1 < NCHUNK:
                fillers, state_next = qkv_steps(i + 1, st_next)
            else:
                # last chunk: feed the g-loop + epilogue with the first FFN
                # half's transposes instead of qkv work
                x2t0, xn2T0, tps0 = ffn0
                fillers = list(tps0)
            attn_both(i, q_cur, y2_sb, fillers, reserve=12 if fillers else 0)
            if i == 1:
                for kk in range(8):
                    nc.sync.dma_start(out=g_sb[:, kk * DI:(kk + 1) * DI],
                                      in_=gT[kk * 128:(kk + 1) * 128, :])
                    nc.sync.dma_start(out=u_sb[:, kk * DI:(kk + 1) * DI],
                                      in_=uT[kk * 128:(kk + 1) * 128, :])

            if i + 2 < NCHUNK:
                st_next = stats(i + 2)
            o_proj(i, y2_sb)
            if i + 1 < NCHUNK:
                q_cur = state_next["q_sb"]
            if i == 5:
                ffn0 = ffn_prep(0)
            if i % 2 == 1:
                c = i // 2
                if no_collective:
                    nc.sync.dma_start(
                        out=rs_out[c * 128:(c + 1) * 128, :],
                        in_=rs_in[c * 1024:c * 1024 + 128, :])
                else:
                    nc.gpsimd.collective_compute(
                        "ReduceScatter", mybir.AluOpType.add,
                        ins=[rs_in[c * 1024:(c + 1) * 1024, :]],
                        outs=[rs_out[c * 128:(c + 1) * 128, :]],
                        replica_groups=[list(range(N_CORES))],
                    )

        x2t1, xn2T1, tps1 = ffn_prep(1)
        ffn_mats(0, x2t0, xn2T0, tps1)
        ffn_mats(1, x2t1, xn2T1)

    ctx.close()


# ===================== host-side sharding =====================

def make_in_maps(x, ln1_w, ln2_w, qkv_w, o_w, gate_w, up_w, down_w,
                 ffn_np_dtype=None):
    import ml_dtypes
    if ffn_np_dtype is None:
        ffn_np_dtype = ml_dtypes.bfloat16
    x = np.asarray(x, np.float32)
    xf = np.ascontiguousarray(x.reshape(NTOK, D))
    xT = np.ascontiguousarray(xf.T).astype(ml_dtypes.bfloat16)

    qkv_eff = np.asarray(qkv_w, np.float32) * np.asarray(ln1_w, np.float32)[None, :]
    g_eff = np.asarray(gate_w, np.float32) * np.asarray(ln2_w, np.float32)[None, :]
    u_eff = np.asarray(up_w, np.float32) * np.asarray(ln2_w, np.float32)[None, :]
    o_w = np.asarray(o_w, np.float32)
    down_w = np.asarray(down_w, np.float32)

    gT = np.ascontiguousarray(g_eff.T).astype(ffn_np_dtype)
    uT = np.ascontiguousarray(u_eff.T).astype(ffn_np_dtype)
    dT = np.ascontiguousarray(down_w.T).astype(ffn_np_dtype)

    scale = 1.0 / np.sqrt(HD)
    in_maps = []
    for r in range(N_CORES):
        hsl = slice(r * HPC * HD, (r + 1) * HPC * HD)  # rows for this core's heads
        qr = qkv_eff[hsl, :] * scale          # [128, D] pre-scaled q
        kr = qkv_eff[D + r * 128:D + (r + 1) * 128, :]
        vr = qkv_eff[2 * D + r * 128:2 * D + (r + 1) * 128, :]
        qkvT_r = np.ascontiguousarray(
            np.concatenate([qr, kr, vr], axis=0).T).astype(ml_dtypes.bfloat16)
        o_wT_r = np.ascontiguousarray(o_w[:, hsl].T).astype(ml_dtypes.bfloat16)
        xo = np.ascontiguousarray(
            xf.reshape(NBLK, 128, D)[r::N_CORES].reshape(512, D))
        in_maps.append({
            "xT": xT, "x_own": xo, "qkvT": qkvT_r, "o_wT": o_wT_r,
            "gT": gT, "uT": uT, "dT": dT,
        })
    return in_maps


def assemble_out(results):
    outf = np.empty((NTOK, D), np.float32)
    for r in range(N_CORES):
        outf.reshape(NBLK, 128, D)[r::N_CORES] = \
            results[r]["out"].reshape(4, 128, D)
    return outf.reshape(B, T, D)


# ===================== entry point =====================

_NC_CACHE = {}


def _get_nc():
    if "nc" not in _NC_CACHE:
        _NC_CACHE["nc"] = build_nc()
    return _NC_CACHE["nc"]


def kernel(x, ln1_w, ln2_w, qkv_w, o_w, gate_w, up_w, down_w):
    from concourse.bass_utils import run_bass_kernel_spmd

    nc = _get_nc()
    in_maps = make_in_maps(x, ln1_w, ln2_w, qkv_w, o_w, gate_w, up_w, down_w)
    res = run_bass_kernel_spmd(nc, in_maps, core_ids=list(range(N_CORES)))
    return assemble_out(res.results)


# revision 30
# speedup vs baseline: 2.3684x; 1.2408x over previous
"""Trainium2 Bass kernel for nn_MiniDecoderBlock (B=2, T=2048, D=1024, H=16, DI=2048).

Strategy: 8-way tensor-parallel attention (2 heads/core, both batches),
one chunked ReduceScatter of the o_proj partial sums distributing tokens,
then token-sharded FFN (512 tokens/core, full d_inner).

kernel(**inputs) takes the FULL unsharded inputs and returns the FULL
output; sharding/compile/run happen inside.

Layout conventions (device side, per core):
  - Activations feature-major: xT [D, tokens] so matmul contraction (partition
    dim) is the feature dim.
  - Scores computed transposed: scoresT [k_tokens(P), q_tokens(free)] so the
    PV matmul uses stationary V and lands yT feature-major for o_proj.
  - V stored token-major with an appended ones column (sumexp for free).
  - rmsnorm applied via a gpsimd broadcast of the rms row onto all partitions,
    multiplied into q/k/v at the mandatory PSUM->SBUF copy.
  - ReduceScatter distributes attention partial sums by token blocks; core r
    owns global 128-token blocks {8c + r}.

Perf notes vs the first working version:
  - Activation tables: Ln/Exp/Square all forced into the combined
    natural_log_exp set (cache surgery on get_activation_tables) so the
    softmax stream never reloads the ACT table; FFN rsqrt runs on DVE
    (Quake-style seed + 2 Newton steps) so only silu ever switches sets.
  - Attention epilogue (recip/broadcast/normalize) is covered by reserved
    PE filler work instead of idling the PE.
  - Scores/PV operands in bf16; o_proj PSUM->SBUF copies on ACT.
  - Bulk DMAs batched with rearrange APs (one per chunk / weight tensor).
  - down_w resident in SBUF.
"""

import numpy as np

import concourse.bass as bass
import concourse.mybir as mybir
import concourse.tile as tile
from concourse import bacc
from concourse.masks import make_identity
from concourse.tile import TileContext

F32 = mybir.dt.float32
F32R = mybir.dt.float32r
I32 = mybir.dt.int32
BF16 = mybir.dt.bfloat16

N_CORES = 8
B, T, D = 2, 2048, 1024
H, HD = 16, 64
DI = 2048
HPC = H // N_CORES          # heads per core = 2
NTOK = B * T                # 4096
NCHUNK = NTOK // 512        # 8 x 512-token chunks
NBLK = NTOK // 128          # 32 x 128-token blocks
EPS = 1e-6
NEG = -1e30


def _patch_act_tables(arch):
    """Collapse Ln/Exp/Square onto the combined natural_log_exp table so the
    compiler never ping-pongs ACT table loads between Ln and Exp sets."""
    try:
        from concourse.hw_specs import get_activation_tables
        A = mybir.ActivationFunctionType
        tabs = get_activation_tables(arch)
        if "natural_log_exp_and_others" not in tabs:
            return
        for nm in tabs:
            if nm == "natural_log_exp_and_others":
                break
            tabs[nm].discard(A.Exp)
            tabs[nm].discard(A.Ln)
            tabs[nm].discard(A.Square)
    except Exception:
        pass


def build_nc(ffn_w_dtype=BF16, reps=1, no_collective=False):
    nc = bacc.Bacc("TRN2", target_bir_lowering=False, debug=False,
                   num_devices=1 if no_collective else N_CORES)
    _patch_act_tables(nc.m.arch)

    xT = nc.dram_tensor("xT", [D, NTOK], BF16, kind="ExternalInput")
    x_own = nc.dram_tensor("x_own", [512, D], BF16, kind="ExternalInput")
    qkvT = nc.dram_tensor("qkvT", [D, 3 * HPC * HD], BF16, kind="ExternalInput")
    o_wT = nc.dram_tensor("o_wT", [HPC * HD, D], BF16, kind="ExternalInput")
    gT = nc.dram_tensor("gT", [D, DI], ffn_w_dtype, kind="ExternalInput")
    uT = nc.dram_tensor("uT", [D, DI], ffn_w_dtype, kind="ExternalInput")
    dT = nc.dram_tensor("dT", [DI, D], ffn_w_dtype, kind="ExternalInput")
    out = nc.dram_tensor("out", [512, D], F32, kind="ExternalOutput")

    with TileContext(nc) as tc:
        emit(nc, tc, xT, x_own, qkvT, o_wT, gT, uT, dT, out, reps=reps,
             no_collective=no_collective)
    nc.compile()
    return nc


def emit(nc, tc, xT, x_own, qkvT, o_wT, gT, uT, dT, out, reps=1, no_collective=False):
    EXP = mybir.ActivationFunctionType.Exp
    LN = mybir.ActivationFunctionType.Ln
    SQUARE = mybir.ActivationFunctionType.Square
    SILU = mybir.ActivationFunctionType.Silu
    COPY = mybir.ActivationFunctionType.Copy
    MUL = mybir.AluOpType.mult
    ADD = mybir.AluOpType.add

    from contextlib import ExitStack
    ctx = ExitStack()
    consts = ctx.enter_context(tc.tile_pool(name="consts", bufs=1))
    dram = ctx.enter_context(tc.tile_pool(name="dram", bufs=1, space="DRAM"))
    psum = ctx.enter_context(tc.tile_pool(name="psum", bufs=2, space="PSUM"))
    sb = ctx.enter_context(tc.tile_pool(name="sb", bufs=2))

    # ---- constants ----
    ident_bf = consts.tile([128, 128], BF16, tag="ident_bf")
    make_identity(nc, ident_bf[:, :])
    ones_c = consts.tile([128, 1], BF16, tag="ones_c")
    nc.vector.memset(ones_c[:, :], 1.0)
    eps_col = consts.tile([128, 1], F32, tag="eps_col")
    nc.vector.memset(eps_col[:, :], EPS)
    # causal mask addend: -BIG where k > q within a 128x128 diagonal block
    madd = consts.tile([128, 128], F32, tag="madd")
    nc.gpsimd.memset(madd[:, :], 0.0)
    nc.gpsimd.affine_select(
        out=madd[:, :], in_=madd[:, :],
        compare_op=mybir.AluOpType.is_ge, fill=NEG,
        base=0, pattern=[[1, 128]], channel_multiplier=-1,
    )

    # ---- persistent SBUF ----
    # startup weight loads go on the ACT queue so chunk-0 xt loads own SP
    qkvT_sb = consts.tile([128, 8 * 384], BF16, tag="qkvT_sb")
    for kk in range(8):
        nc.scalar.dma_start(out=qkvT_sb[:, kk * 384:(kk + 1) * 384],
                            in_=qkvT[kk * 128:(kk + 1) * 128, :])
    o_wT_sb = consts.tile([128, D], BF16, tag="o_wT_sb")
    nc.scalar.dma_start(out=o_wT_sb[:, :], in_=o_wT[:, :])

    kT_all = consts.tile([128, NTOK], BF16, tag="kT_all")
    v_aug = consts.tile([128, HPC * NBLK * 65], BF16, tag="v_aug")
    nc.vector.memset(v_aug[:, :], 1.0)

    # FFN weights resident, bf16 -- loaded during early chunks
    g_sb = consts.tile([128, 8 * DI], gT.dtype, tag="g_sb")
    u_sb = consts.tile([128, 8 * DI], uT.dtype, tag="u_sb")
    d_sb = consts.tile([128, 16 * D], dT.dtype, tag="d_sb")

    # ---- DRAM bounce ----
    rs_in = dram.tile([NTOK, D], BF16, tag="rs_in")
    rs_out = dram.tile([512, D], BF16, tag="rs_out")

    for _rep in range(reps):
        # ================= main loop over 512-token chunks =================
        def stats(i):
            """Load xT chunk i + rms broadcast tile (emitted ~1.5 chunks ahead)."""
            csl = slice(i * 512, (i + 1) * 512)
            xt = sb.tile([128, 8 * 512], BF16, tag="xt", name=f"xt_{i}")
            for kk in range(8):
                nc.sync.dma_start(out=xt[:, kk * 512:(kk + 1) * 512],
                                  in_=xT[kk * 128:(kk + 1) * 128, csl])
            ss = psum.tile([1, 512], F32, tag="proj", bufs=3, name=f"ss_{i}")
            for kk in range(8):
                sq = sb.tile([128, 512], BF16, tag="sq", bufs=2,
                             name=f"sq_{i}_{kk}")
                nc.vector.tensor_tensor(out=sq[:, :],
                                        in0=xt[:, kk * 512:(kk + 1) * 512],
                                        in1=xt[:, kk * 512:(kk + 1) * 512],
                                        op=MUL)
                nc.tensor.matmul(ss[:, :], ones_c[:, :], sq[:, :],
                                 start=(kk == 0), stop=(kk == 7))
            lt = sb.tile([1, 512], F32, tag="lt", bufs=1, name=f"lt_{i}")
            nc.scalar.activation(lt[:, :], ss[:, :], LN,
                                 bias=eps_col[0:1, :], scale=1.0 / D)
            rms_row = sb.tile([1, 512], BF16, tag="rms_row", name=f"rmsr_{i}")
            nc.scalar.activation(rms_row[:, :], lt[:, :], EXP, scale=-0.5)
            rms_b = sb.tile([128, 512], BF16, tag="rms_b", name=f"rmsb_{i}")
            nc.gpsimd.partition_broadcast(rms_b[:, :], rms_row[0:1, :])
            return xt, rms_b

        def qkv_steps(i, st):
            """Projection for chunk i as filler closures sprinkled into the
            previous chunk's attention g-loop (PE fills exp-wait gaps)."""
            csl = slice(i * 512, (i + 1) * 512)
            xt, rms_b = st
            state = {}
            steps = []

            def mk_proj(w, off, kk):
                def f():
                    if kk == 0:
                        state[w] = psum.tile([128, 512], F32, tag="proj",
                                             bufs=3, name=f"pj_{w}_{i}")
                    nc.tensor.matmul(
                        state[w][:, :],
                        qkvT_sb[:, kk * 384 + off:kk * 384 + off + 128],
                        xt[:, kk * 512:(kk + 1) * 512],
                        start=(kk == 0), stop=(kk == 7))
                return f

            for w, off in (("q", 0), ("k", 128), ("v", 256)):
                for kk in range(8):
                    steps.append(mk_proj(w, off, kk))

            def mk_qk_epi():
                def f():
                    q_sb = sb.tile([128, 512], BF16, tag="q_sb", name=f"q_{i}")
                    state["q_sb"] = q_sb
                    nc.vector.tensor_tensor(out=q_sb[:, :], in0=state["q"][:, :],
                                            in1=rms_b[:, :], op=MUL)
                    nc.vector.tensor_tensor(out=kT_all[:, csl],
                                            in0=state["k"][:, :],
                                            in1=rms_b[:, :], op=MUL)
                    v_sb = sb.tile([128, 512], BF16, tag="v_sb", name=f"v_{i}")
                    state["v_sb"] = v_sb
                    nc.vector.tensor_tensor(out=v_sb[:, :], in0=state["v"][:, :],
                                            in1=rms_b[:, :], op=MUL)
                return f

            steps.append(mk_qk_epi())

            def mk_vt(h, j):
                def f():
                    gb = i * 4 + j
                    v_sb = state["v_sb"]
                    vt = psum.tile([128, 64], BF16, tag="proj", bufs=3)
                    nc.tensor.transpose(vt[:, :],
                                        v_sb[h * 64:(h + 1) * 64,
                                             j * 128:(j + 1) * 128],
                                        ident_bf[h * 64:(h + 1) * 64,
                                                  h * 64:(h + 1) * 64])
                    slot = (h * NBLK + gb) * 65
                    nc.vector.tensor_copy(v_aug[:, slot:slot + 64], vt[:, :])
                return f

            for h in range(HPC):
                for j in range(4):
                    steps.append(mk_vt(h, j))
            return steps, state

        def attn_both(i, q_sb, y2_sb, fillers=(), reserve=0):
            """Scores+softmax+PV for chunk i, then the normalize epilogue.
            Keeps `reserve` fillers back to feed the PE during the epilogue's
            DVE/Pool dependency chain."""
            fillers = list(fillers)
            b, li = divmod(i, 4)
            nblk = li * 4 + 4
            avail = max(0, len(fillers) - reserve)
            per_g = max(1, -(-avail // max(1, nblk)))
            yT = [psum.tile([65, 512], F32, tag="yT", bufs=2, name=f"yT_{i}_{h}")
                  for h in range(2)]
            for g in range(nblk):
                gb = b * 16 + g
                q_off = max(0, g - li * 4) * 128
                w = 512 - q_off
                scs = []
                for h in range(2):
                    sc = psum.tile([128, 512], F32, tag="sc", bufs=3,
                                   name=f"sc{h}")
                    # lhsT at partitions h*64..h*64+64 -> distinct PE row
                    # groups; the two matmuls run concurrently in the array.
                    nc.tensor.matmul(
                        sc[:, 0:w],
                        kT_all[h * 64:(h + 1) * 64, gb * 128:(gb + 1) * 128],
                        q_sb[h * 64:(h + 1) * 64, q_off:512],
                        start=True, stop=True)
                    scs.append(sc)
                for h in range(2):
                    sc = scs[h]
                    if g >= li * 4:
                        nc.vector.tensor_tensor(out=sc[:, 0:128],
                                                in0=sc[:, 0:128],
                                                in1=madd[:, :], op=ADD)
                    pT = sb.tile([128, 512], BF16, tag="pT", bufs=3,
                                 name=f"pT{h}")
                    nc.scalar.activation(pT[:, 0:w], sc[:, 0:w], EXP)
                    slot = (h * NBLK + gb) * 65
                    nc.tensor.matmul(
                        yT[h][:, q_off:512],
                        v_aug[:, slot:slot + 65],
                        pT[:, 0:w],
                        start=(g == 0), stop=(g == nblk - 1))
                for _ in range(per_g):
                    if len(fillers) > reserve:
                        fillers.pop(0)()
            # normalize epilogue: reciprocal of the sumexp row (direct from
            # PSUM), broadcast, apply -- the reserved fillers keep PE fed.
            for h in range(2):
                rec = sb.tile([1, 512], F32, tag="rec")
                nc.vector.reciprocal(rec[:, :], yT[h][64:65, :])
                rb = sb.tile([64, 512], F32, tag=f"rb{h}", bufs=1)
                nc.gpsimd.partition_broadcast(rb[:, :], rec[0:1, :])
                nc.vector.tensor_tensor(out=y2_sb[h * 64:(h + 1) * 64, :],
                                        in0=yT[h][0:64, :], in1=rb[:, :],
                                        op=MUL)
            while fillers:
                fillers.pop(0)()

        def o_proj(i, y2_sb):
            osb = sb.tile([128, 4 * D], BF16, tag="osb", bufs=1)
            for j in range(4):
                for n in range(2):
                    op = psum.tile([128, 512], F32, tag="sc", bufs=3)
                    nc.tensor.matmul(op[:, :],
                                     y2_sb[:, j * 128:(j + 1) * 128],
                                     o_wT_sb[:, n * 512:(n + 1) * 512],
                                     start=True, stop=True)
                    nc.scalar.activation(
                        osb[:, j * D + n * 512:j * D + (n + 1) * 512],
                        op[:, :], COPY)
            for j in range(4):
                r0 = i * 512 + j * 128
                nc.sync.dma_start(out=rs_in[r0:r0 + 128, :],
                                  in_=osb[:, j * D:(j + 1) * D])

        # ================= FFN on own 512 tokens =================
        def dve_rsqrt(v, tag):
            """y = rsqrt(v) on DVE: Quake seed + 2 Newton steps. v: [128,1] f32."""
            y = sb.tile([128, 1], F32, tag=f"{tag}y", name=f"{tag}y")
            a = sb.tile([128, 1], F32, tag=f"{tag}a", name=f"{tag}a")
            # seed: y_i = 0x5f3759df - (v_i >> 1)  (via xor/add trick)
            nc.vector.tensor_scalar(
                out=a[:, :].bitcast(I32), in0=v[:, :].bitcast(I32),
                scalar1=1, scalar2=None,
                op0=mybir.AluOpType.arith_shift_right)
            nc.vector.tensor_scalar(
                out=y[:, :].bitcast(I32), in0=a[:, :].bitcast(I32),
                scalar1=-1, scalar2=None,
                op0=mybir.AluOpType.bitwise_xor)
            nc.vector.tensor_scalar(
                out=y[:, :].bitcast(I32), in0=y[:, :].bitcast(I32),
                scalar1=0x5f3759e0, scalar2=None, op0=ADD)
            for _ in range(2):
                nc.vector.tensor_tensor(out=a[:, :], in0=v[:, :], in1=y[:, :],
                                        op=MUL)
                nc.vector.tensor_tensor(out=a[:, :], in0=a[:, :], in1=y[:, :],
                                        op=MUL)
                nc.vector.tensor_scalar(
                    out=a[:, :], in0=a[:, :], scalar1=-0.5, scalar2=1.5,
                    op0=MUL, op1=ADD)
                nc.vector.tensor_tensor(out=y[:, :], in0=y[:, :], in1=a[:, :],
                                        op=MUL)
            return y

        def ffn_prep(ha):
            x2t, xn2T, xn2s = [], [], []
            for jj in range(2):
                c2 = ha * 2 + jj
                rsx = sb.tile([128, D], BF16, tag="rsx", bufs=1, name=f"rsx{c2}")
                nc.sync.dma_start(out=rsx[:, :],
                                  in_=rs_out[c2 * 128:(c2 + 1) * 128, :])
                xo = sb.tile([128, D], BF16, tag="xo", bufs=1, name=f"xo{c2}")
                nc.sync.dma_start(out=xo[:, :],
                                  in_=x_own[c2 * 128:(c2 + 1) * 128, :])
                x2 = sb.tile([128, D], BF16, tag=f"x2_{jj}", bufs=2,
                             name=f"x2_{c2}")
                nc.vector.tensor_tensor(out=x2[:, :], in0=rsx[:, :],
                                        in1=xo[:, :], op=ADD)
                x2t.append(x2)
                # rsx is dead after the x2 add; reuse it as the square scratch
                ss2 = sb.tile([128, 1], F32, tag="ss2", name=f"ss2_{c2}")
                nc.scalar.activation(rsx[:, :], x2[:, :], SQUARE,
                                     accum_out=ss2[:, :])
                t2 = sb.tile([128, 1], F32, tag="t2", name=f"t2_{c2}")
                nc.vector.tensor_scalar(
                    out=t2[:, :], in0=ss2[:, :], scalar1=1.0 / D, scalar2=EPS,
                    op0=MUL, op1=ADD)
                r2 = dve_rsqrt(t2, tag=f"r2_{c2}")
                xn2 = sb.tile([128, D], BF16, tag=f"xn2_{jj}", bufs=1,
                              name=f"xn2_{c2}")
                nc.vector.tensor_scalar_mul(xn2[:, :], x2[:, :], r2[:, :])
                xn2s.append(xn2)
            for kk in range(8):
                xt2 = sb.tile([128, 256], gT.dtype, tag=f"xn2T{kk}",
                              bufs=2, name=f"xn2T{kk}_{ha}")
                xn2T.append(xt2)

            def mk_tp(jj, kk):
                def f():
                    xp = psum.tile([128, 128], BF16, tag="proj", bufs=3)
                    nc.tensor.transpose(xp[:, :],
                                        xn2s[jj][:, kk * 128:(kk + 1) * 128],
                                        ident_bf[:, :])
                    nc.vector.tensor_copy(xn2T[kk][:, jj * 128:(jj + 1) * 128],
                                          xp[:, :])
                return f

            tps = [mk_tp(jj, kk) for jj in range(2) for kk in range(8)]
            return x2t, xn2T, tps

        def ffn_mats(ha, x2t, xn2T, fillers=()):
            fillers = list(fillers)
            h_sb = []
            for m in range(16):
                if m >= 8 and fillers:
                    fillers.pop(0)()
                    if fillers:
                        fillers.pop(0)()
                gp = psum.tile([128, 256], F32, tag="sc", bufs=3)
                up = psum.tile([128, 256], F32, tag="sc", bufs=3)
                for kk in range(8):
                    nc.tensor.matmul(gp[:, :],
                                     g_sb[:, kk * DI + m * 128:kk * DI + (m + 1) * 128],
                                     xn2T[kk][:, :],
                                     start=(kk == 0), stop=(kk == 7))
                for kk in range(8):
                    nc.tensor.matmul(up[:, :],
                                     u_sb[:, kk * DI + m * 128:kk * DI + (m + 1) * 128],
                                     xn2T[kk][:, :],
                                     start=(kk == 0), stop=(kk == 7))
                sg = sb.tile([128, 256], BF16, tag="sg")
                nc.scalar.activation(sg[:, :], gp[:, :], SILU)
                hm = sb.tile([128, 256], dT.dtype, tag=f"h{m}", bufs=1,
                             name=f"h{m}_{ha}")
                nc.vector.tensor_tensor(out=hm[:, :], in0=sg[:, :],
                                        in1=up[:, :], op=MUL)
                h_sb.append(hm)

            while fillers:
                fillers.pop(0)()
            dp = [psum.tile([128, 512], F32,
                            tag="proj" if nn == 0 else "sc", bufs=3,
                            name=f"dp{jj}_{nn}")
                  for nn in range(2) for jj in range(2)]
            for m in range(16):
                for n in range(2):
                    for jj in range(2):
                        nc.tensor.matmul(
                            dp[n * 2 + jj][:, :],
                            h_sb[m][:, jj * 128:(jj + 1) * 128],
                            d_sb[:, m * D + n * 512:m * D + (n + 1) * 512],
                            start=(m == 0), stop=(m == 15))
            for jj in range(2):
                c2 = ha * 2 + jj
                osb = sb.tile([128, D], F32, tag="fout", bufs=1)
                for n in range(2):
                    nc.vector.tensor_tensor(out=osb[:, n * 512:(n + 1) * 512],
                                            in0=dp[n * 2 + jj][:, :],
                                            in1=x2t[jj][:, n * 512:(n + 1) * 512],
                                            op=ADD)
                nc.sync.dma_start(out=out[c2 * 128:(c2 + 1) * 128, :],
                                  in_=osb[:, :])


        st = stats(0)
        steps0, state0 = qkv_steps(0, st)
        for f in steps0:
            f()
        q_cur = state0["q_sb"]
        st_next = stats(1)
        state_next = None
        ffn0 = None
        for i in range(NCHUNK):
            y2_sb = sb.tile([128, 512], BF16, tag="y2_sb", name=f"y2_{i}")
            if i + 1 < NCHUNK:
                fillers, state_next = qkv_steps(i + 1, st_next)
            else:
                # last chunk: feed the g-loop + epilogue with the first FFN
                # half's transposes instead of qkv work
                x2t0, xn2T0, tps0 = ffn0
                fillers = list(tps0)
            attn_both(i, q_cur, y2_sb, fillers, reserve=12 if fillers else 0)
            if i == 1:
                for kk in range(8):
                    nc.sync.dma_start(out=g_sb[:, kk * DI:(kk + 1) * DI],
                                      in_=gT[kk * 128:(kk + 1) * 128, :])
                    nc.sync.dma_start(out=u_sb[:, kk * DI:(kk + 1) * DI],
                                      in_=uT[kk * 128:(kk + 1) * 128, :])
            if i in (2, 3):
                for m in range((i - 2) * 8, (i - 1) * 8):
                    nc.sync.dma_start(out=d_sb[:, m * D:(m + 1) * D],
                                      in_=dT[m * 128:(m + 1) * 128, :])

            if i + 2 < NCHUNK:
                st_next = stats(i + 2)
            o_proj(i, y2_sb)
            if i + 1 < NCHUNK:
                q_cur = state_next["q_sb"]
            if i == 5:
                ffn0 = ffn_prep(0)
            if i % 2 == 1:
                c = i // 2
                if no_collective:
                    nc.sync.dma_start(
                        out=rs_out[c * 128:(c + 1) * 128, :],
                        in_=rs_in[c * 1024:c * 1024 + 128, :])
                else:
                    nc.gpsimd.collective_compute(
                        "ReduceScatter", mybir.AluOpType.add,
                        ins=[rs_in[c * 1024:(c + 1) * 1024, :]],
                        outs=[rs_out[c * 128:(c + 1) * 128, :]],
                        replica_groups=[list(range(N_CORES))],
                    )

        x2t1, xn2T1, tps1 = ffn_prep(1)
        ffn_mats(0, x2t0, xn2T0, tps1)
        ffn_mats(1, x2t1, xn2T1)

    ctx.close()


# ===================== host-side sharding =====================

def make_in_maps(x, ln1_w, ln2_w, qkv_w, o_w, gate_w, up_w, down_w,
                 ffn_np_dtype=None):
    import ml_dtypes
    if ffn_np_dtype is None:
        ffn_np_dtype = ml_dtypes.bfloat16
    x = np.asarray(x, np.float32)
    xf = np.ascontiguousarray(x.reshape(NTOK, D))
    xT = np.ascontiguousarray(xf.T).astype(ml_dtypes.bfloat16)

    qkv_eff = np.asarray(qkv_w, np.float32) * np.asarray(ln1_w, np.float32)[None, :]
    g_eff = np.asarray(gate_w, np.float32) * np.asarray(ln2_w, np.float32)[None, :]
    u_eff = np.asarray(up_w, np.float32) * np.asarray(ln2_w, np.float32)[None, :]
    o_w = np.asarray(o_w, np.float32)
    down_w = np.asarray(down_w, np.float32)

    gT = np.ascontiguousarray(g_eff.T).astype(ffn_np_dtype)
    uT = np.ascontiguousarray(u_eff.T).astype(ffn_np_dtype)
    dT = np.ascontiguousarray(down_w.T).astype(ffn_np_dtype)

    scale = 1.0 / np.sqrt(HD)
    in_maps = []
    for r in range(N_CORES):
        hsl = slice(r * HPC * HD, (r + 1) * HPC * HD)  # rows for this core's heads
        qr = qkv_eff[hsl, :] * scale          # [128, D] pre-scaled q
        kr = qkv_eff[D + r * 128:D + (r + 1) * 128, :]
        vr = qkv_eff[2 * D + r * 128:2 * D + (r + 1) * 128, :]
        qkvT_r = np.ascontiguousarray(
            np.concatenate([qr, kr, vr], axis=0).T).astype(ml_dtypes.bfloat16)
        o_wT_r = np.ascontiguousarray(o_w[:, hsl].T).astype(ml_dtypes.bfloat16)
        xo = np.ascontiguousarray(
            xf.reshape(NBLK, 128, D)[r::N_CORES].reshape(512, D)).astype(
                ml_dtypes.bfloat16)
        in_maps.append({
            "xT": xT, "x_own": xo, "qkvT": qkvT_r, "o_wT": o_wT_r,
            "gT": gT, "uT": uT, "dT": dT,
        })
    return in_maps


def assemble_out(results):
    outf = np.empty((NTOK, D), np.float32)
    for r in range(N_CORES):
        outf.reshape(NBLK, 128, D)[r::N_CORES] = \
            results[r]["out"].reshape(4, 128, D)
    return outf.reshape(B, T, D)


# ===================== entry point =====================

_NC_CACHE = {}


def _get_nc():
    if "nc" not in _NC_CACHE:
        _NC_CACHE["nc"] = build_nc()
    return _NC_CACHE["nc"]


def kernel(x, ln1_w, ln2_w, qkv_w, o_w, gate_w, up_w, down_w):
    from concourse.bass_utils import run_bass_kernel_spmd

    nc = _get_nc()
    in_maps = make_in_maps(x, ln1_w, ln2_w, qkv_w, o_w, gate_w, up_w, down_w)
    res = run_bass_kernel_spmd(nc, in_maps, core_ids=list(range(N_CORES)))
    return assemble_out(res.results)


# revision 33
# speedup vs baseline: 2.5005x; 1.0558x over previous
"""Trainium2 Bass kernel for nn_MiniDecoderBlock (B=2, T=2048, D=1024, H=16, DI=2048).

Strategy: 8-way tensor-parallel attention (2 heads/core, both batches),
one chunked ReduceScatter of the o_proj partial sums distributing tokens,
then token-sharded FFN (512 tokens/core, full d_inner).

kernel(**inputs) takes the FULL unsharded inputs and returns the FULL
output; sharding/compile/run happen inside.

Layout conventions (device side, per core):
  - Activations feature-major: xT [D, tokens] so matmul contraction (partition
    dim) is the feature dim.
  - Scores computed transposed: scoresT [k_tokens(P), q_tokens(free)] so the
    PV matmul uses stationary V and lands yT feature-major for o_proj.
  - V stored token-major with an appended ones column (sumexp for free).
  - rmsnorm applied via a gpsimd broadcast of the rms row onto all partitions,
    multiplied into q/k/v at the mandatory PSUM->SBUF copy.
  - ReduceScatter distributes attention partial sums by token blocks; core r
    owns global 128-token blocks {8c + r}.

Perf notes vs the first working version:
  - Activation tables: Ln/Exp/Square all forced into the combined
    natural_log_exp set (cache surgery on get_activation_tables) so the
    softmax stream never reloads the ACT table; FFN rsqrt runs on DVE
    (Quake-style seed + 2 Newton steps) so only silu ever switches sets.
  - Attention epilogue (recip/broadcast/normalize) is covered by reserved
    PE filler work instead of idling the PE.
  - Scores/PV operands in bf16; o_proj PSUM->SBUF copies on ACT.
  - Bulk DMAs batched with rearrange APs (one per chunk / weight tensor).
  - down_w resident in SBUF.
"""

import numpy as np

import concourse.bass as bass
import concourse.mybir as mybir
import concourse.tile as tile
from concourse import bacc
from concourse.masks import make_identity
from concourse.tile import TileContext

F32 = mybir.dt.float32
F32R = mybir.dt.float32r
I32 = mybir.dt.int32
BF16 = mybir.dt.bfloat16

N_CORES = 8
B, T, D = 2, 2048, 1024
H, HD = 16, 64
DI = 2048
HPC = H // N_CORES          # heads per core = 2
NTOK = B * T                # 4096
NCHUNK = NTOK // 512        # 8 x 512-token chunks
NBLK = NTOK // 128          # 32 x 128-token blocks
EPS = 1e-6
NEG = -1e30


def _patch_act_tables(arch):
    """Collapse Ln/Exp/Square onto the combined natural_log_exp table so the
    compiler never ping-pongs ACT table loads between Ln and Exp sets."""
    try:
        from concourse.hw_specs import get_activation_tables
        A = mybir.ActivationFunctionType
        tabs = get_activation_tables(arch)
        if "natural_log_exp_and_others" not in tabs:
            return
        for nm in tabs:
            if nm == "natural_log_exp_and_others":
                break
            tabs[nm].discard(A.Exp)
            tabs[nm].discard(A.Ln)
            tabs[nm].discard(A.Square)
    except Exception:
        pass


def build_nc(ffn_w_dtype=BF16, reps=1, no_collective=False):
    nc = bacc.Bacc("TRN2", target_bir_lowering=False, debug=False,
                   num_devices=1 if no_collective else N_CORES)
    _patch_act_tables(nc.m.arch)

    xT = nc.dram_tensor("xT", [D, NTOK], BF16, kind="ExternalInput")
    x_own = nc.dram_tensor("x_own", [512, D], BF16, kind="ExternalInput")
    qkvT = nc.dram_tensor("qkvT", [D, 3 * HPC * HD], BF16, kind="ExternalInput")
    o_wT = nc.dram_tensor("o_wT", [HPC * HD, D], BF16, kind="ExternalInput")
    gT = nc.dram_tensor("gT", [D, DI], ffn_w_dtype, kind="ExternalInput")
    uT = nc.dram_tensor("uT", [D, DI], ffn_w_dtype, kind="ExternalInput")
    dT = nc.dram_tensor("dT", [DI, D], ffn_w_dtype, kind="ExternalInput")
    out = nc.dram_tensor("out", [512, D], F32, kind="ExternalOutput")

    with TileContext(nc) as tc:
        emit(nc, tc, xT, x_own, qkvT, o_wT, gT, uT, dT, out, reps=reps,
             no_collective=no_collective)
    nc.compile()
    return nc


def emit(nc, tc, xT, x_own, qkvT, o_wT, gT, uT, dT, out, reps=1, no_collective=False):
    EXP = mybir.ActivationFunctionType.Exp
    LN = mybir.ActivationFunctionType.Ln
    SQUARE = mybir.ActivationFunctionType.Square
    SILU = mybir.ActivationFunctionType.Silu
    COPY = mybir.ActivationFunctionType.Copy
    MUL = mybir.AluOpType.mult
    ADD = mybir.AluOpType.add

    from contextlib import ExitStack
    ctx = ExitStack()
    consts = ctx.enter_context(tc.tile_pool(name="consts", bufs=1))
    dram = ctx.enter_context(tc.tile_pool(name="dram", bufs=1, space="DRAM"))
    psum = ctx.enter_context(tc.tile_pool(name="psum", bufs=2, space="PSUM"))
    sb = ctx.enter_context(tc.tile_pool(name="sb", bufs=2))

    # ---- constants ----
    ident_bf = consts.tile([128, 128], BF16, tag="ident_bf")
    make_identity(nc, ident_bf[:, :])
    ones_c = consts.tile([128, 1], BF16, tag="ones_c")
    nc.vector.memset(ones_c[:, :], 1.0)
    eps_col = consts.tile([128, 1], F32, tag="eps_col")
    nc.vector.memset(eps_col[:, :], EPS)
    # causal mask addend: -BIG where k > q within a 128x128 diagonal block
    madd = consts.tile([128, 128], F32, tag="madd")
    nc.gpsimd.memset(madd[:, :], 0.0)
    nc.gpsimd.affine_select(
        out=madd[:, :], in_=madd[:, :],
        compare_op=mybir.AluOpType.is_ge, fill=NEG,
        base=0, pattern=[[1, 128]], channel_multiplier=-1,
    )

    # ---- persistent SBUF ----
    # startup weight loads go on the ACT queue so chunk-0 xt loads own SP
    qkvT_sb = consts.tile([128, 8 * 384], BF16, tag="qkvT_sb")
    for kk in range(8):
        nc.scalar.dma_start(out=qkvT_sb[:, kk * 384:(kk + 1) * 384],
                            in_=qkvT[kk * 128:(kk + 1) * 128, :])
    o_wT_sb = consts.tile([128, D], BF16, tag="o_wT_sb")
    nc.scalar.dma_start(out=o_wT_sb[:, :], in_=o_wT[:, :])

    kT_all = consts.tile([128, NTOK], BF16, tag="kT_all")
    v_aug = consts.tile([128, HPC * NBLK * 65], BF16, tag="v_aug")
    nc.vector.memset(v_aug[:, :], 1.0)

    # FFN weights resident, bf16 -- loaded during early chunks
    g_sb = consts.tile([128, 8 * DI], gT.dtype, tag="g_sb")
    u_sb = consts.tile([128, 8 * DI], uT.dtype, tag="u_sb")
    d_sb = consts.tile([128, 16 * D], dT.dtype, tag="d_sb")

    # ---- DRAM bounce ----
    rs_in = dram.tile([NTOK, D], BF16, tag="rs_in")
    rs_out = dram.tile([512, D], BF16, tag="rs_out")

    # PE warm-up: dependency-free dummy matmuls run during the initial DMA
    # wait so the p-state ramp (3.4us activity window) completes before the
    # first real matmul.
    warm = psum.tile([128, 128], F32, tag="sc", bufs=3)
    for _ in range(40):
        nc.tensor.matmul(warm[:, :], ident_bf[:, :], ident_bf[:, :],
                         start=True, stop=True)

    for _rep in range(reps):
        # ================= main loop over 512-token chunks =================
        def stats(i):
            """Load xT chunk i + rms broadcast tile (emitted ~1.5 chunks ahead)."""
            csl = slice(i * 512, (i + 1) * 512)
            xt = sb.tile([128, 8 * 512], BF16, tag="xt", name=f"xt_{i}")
            if i < 2:
                # startup chunks: halve per-DMA transfer so the first
                # squares/matmuls can begin sooner
                for kk in range(8):
                    for hh in range(2):
                        nc.sync.dma_start(
                            out=xt[:, kk * 512 + hh * 256:
                                   kk * 512 + (hh + 1) * 256],
                            in_=xT[kk * 128:(kk + 1) * 128,
                                   i * 512 + hh * 256:i * 512 + (hh + 1) * 256])
            else:
                for kk in range(8):
                    nc.sync.dma_start(out=xt[:, kk * 512:(kk + 1) * 512],
                                      in_=xT[kk * 128:(kk + 1) * 128, csl])
            ss = psum.tile([1, 512], F32, tag="proj", bufs=3, name=f"ss_{i}")
            for kk in range(8):
                sq = sb.tile([128, 512], BF16, tag="sq", bufs=2,
                             name=f"sq_{i}_{kk}")
                nc.vector.tensor_tensor(out=sq[:, :],
                                        in0=xt[:, kk * 512:(kk + 1) * 512],
                                        in1=xt[:, kk * 512:(kk + 1) * 512],
                                        op=MUL)
                nc.tensor.matmul(ss[:, :], ones_c[:, :], sq[:, :],
                                 start=(kk == 0), stop=(kk == 7))
            lt = sb.tile([1, 512], F32, tag="lt", bufs=1, name=f"lt_{i}")
            nc.scalar.activation(lt[:, :], ss[:, :], LN,
                                 bias=eps_col[0:1, :], scale=1.0 / D)
            rms_row = sb.tile([1, 512], BF16, tag="rms_row", name=f"rmsr_{i}")
            nc.scalar.activation(rms_row[:, :], lt[:, :], EXP, scale=-0.5)
            rms_b = sb.tile([128, 512], BF16, tag="rms_b", name=f"rmsb_{i}")
            nc.gpsimd.partition_broadcast(rms_b[:, :], rms_row[0:1, :])
            return xt, rms_b

        def qkv_steps(i, st):
            """Projection for chunk i as filler closures sprinkled into the
            previous chunk's attention g-loop (PE fills exp-wait gaps)."""
            csl = slice(i * 512, (i + 1) * 512)
            xt, rms_b = st
            state = {}
            steps = []

            def mk_proj(w, off, kk):
                def f():
                    if kk == 0:
                        state[w] = psum.tile([128, 512], F32, tag="proj",
                                             bufs=3, name=f"pj_{w}_{i}")
                    nc.tensor.matmul(
                        state[w][:, :],
                        qkvT_sb[:, kk * 384 + off:kk * 384 + off + 128],
                        xt[:, kk * 512:(kk + 1) * 512],
                        start=(kk == 0), stop=(kk == 7))
                return f

            for w, off in (("q", 0), ("k", 128), ("v", 256)):
                for kk in range(8):
                    steps.append(mk_proj(w, off, kk))

            def mk_qk_epi():
                def f():
                    q_sb = sb.tile([128, 512], BF16, tag="q_sb", name=f"q_{i}")
                    state["q_sb"] = q_sb
                    nc.vector.tensor_tensor(out=q_sb[:, :], in0=state["q"][:, :],
                                            in1=rms_b[:, :], op=MUL)
                    nc.vector.tensor_tensor(out=kT_all[:, csl],
                                            in0=state["k"][:, :],
                                            in1=rms_b[:, :], op=MUL)
                    v_sb = sb.tile([128, 512], BF16, tag="v_sb", name=f"v_{i}")
                    state["v_sb"] = v_sb
                    nc.vector.tensor_tensor(out=v_sb[:, :], in0=state["v"][:, :],
                                            in1=rms_b[:, :], op=MUL)
                return f

            steps.append(mk_qk_epi())

            def mk_vt(h, j):
                def f():
                    gb = i * 4 + j
                    v_sb = state["v_sb"]
                    vt = psum.tile([128, 64], BF16, tag="proj", bufs=3)
                    nc.tensor.transpose(vt[:, :],
                                        v_sb[h * 64:(h + 1) * 64,
                                             j * 128:(j + 1) * 128],
                                        ident_bf[h * 64:(h + 1) * 64,
                                                  h * 64:(h + 1) * 64])
                    slot = (h * NBLK + gb) * 65
                    nc.vector.tensor_copy(v_aug[:, slot:slot + 64], vt[:, :])
                return f

            for h in range(HPC):
                for j in range(4):
                    steps.append(mk_vt(h, j))
            return steps, state

        def attn_both(i, q_sb, y2_sb, fillers=(), reserve=0):
            """Scores+softmax+PV for chunk i, then the normalize epilogue.
            Keeps `reserve` fillers back to feed the PE during the epilogue's
            DVE/Pool dependency chain."""
            fillers = list(fillers)
            b, li = divmod(i, 4)
            nblk = li * 4 + 4
            avail = max(0, len(fillers) - reserve)
            per_g = max(1, -(-avail // max(1, nblk)))
            yT = [psum.tile([65, 512], F32, tag="yT", bufs=2, name=f"yT_{i}_{h}")
                  for h in range(2)]
            for g in range(nblk):
                gb = b * 16 + g
                q_off = max(0, g - li * 4) * 128
                w = 512 - q_off
                scs = []
                for h in range(2):
                    sc = psum.tile([128, 512], F32, tag="sc", bufs=3,
                                   name=f"sc{h}")
                    # lhsT at partitions h*64..h*64+64 -> distinct PE row
                    # groups; the two matmuls run concurrently in the array.
                    nc.tensor.matmul(
                        sc[:, 0:w],
                        kT_all[h * 64:(h + 1) * 64, gb * 128:(gb + 1) * 128],
                        q_sb[h * 64:(h + 1) * 64, q_off:512],
                        start=True, stop=True)
                    scs.append(sc)
                for h in range(2):
                    sc = scs[h]
                    if g >= li * 4:
                        nc.vector.tensor_tensor(out=sc[:, 0:128],
                                                in0=sc[:, 0:128],
                                                in1=madd[:, :], op=ADD)
                    pT = sb.tile([128, 512], BF16, tag="pT", bufs=3,
                                 name=f"pT{h}")
                    nc.scalar.activation(pT[:, 0:w], sc[:, 0:w], EXP)
                    slot = (h * NBLK + gb) * 65
                    nc.tensor.matmul(
                        yT[h][:, q_off:512],
                        v_aug[:, slot:slot + 65],
                        pT[:, 0:w],
                        start=(g == 0), stop=(g == nblk - 1))
                for _ in range(per_g):
                    if len(fillers) > reserve:
                        fillers.pop(0)()
            # normalize epilogue: reciprocal of the sumexp row (direct from
            # PSUM), broadcast, apply -- the reserved fillers keep PE fed.
            for h in range(2):
                rec = sb.tile([1, 512], F32, tag="rec")
                nc.vector.reciprocal(rec[:, :], yT[h][64:65, :])
                rb = sb.tile([64, 512], F32, tag=f"rb{h}", bufs=1)
                nc.gpsimd.partition_broadcast(rb[:, :], rec[0:1, :])
                nc.vector.tensor_tensor(out=y2_sb[h * 64:(h + 1) * 64, :],
                                        in0=yT[h][0:64, :], in1=rb[:, :],
                                        op=MUL)
            while fillers:
                fillers.pop(0)()

        def o_proj(i, y2_sb):
            osb = sb.tile([128, 4 * D], BF16, tag="osb", bufs=1)
            for j in range(4):
                for n in range(2):
                    op = psum.tile([128, 512], F32, tag="sc", bufs=3)
                    nc.tensor.matmul(op[:, :],
                                     y2_sb[:, j * 128:(j + 1) * 128],
                                     o_wT_sb[:, n * 512:(n + 1) * 512],
                                     start=True, stop=True)
                    nc.scalar.activation(
                        osb[:, j * D + n * 512:j * D + (n + 1) * 512],
                        op[:, :], COPY)
            for j in range(4):
                r0 = i * 512 + j * 128
                nc.sync.dma_start(out=rs_in[r0:r0 + 128, :],
                                  in_=osb[:, j * D:(j + 1) * D])

        # ================= FFN on own 512 tokens =================
        def dve_rsqrt(v, tag):
            """y = rsqrt(v) on DVE: Quake seed + 2 Newton steps. v: [128,1] f32."""
            y = sb.tile([128, 1], F32, tag=f"{tag}y", name=f"{tag}y")
            a = sb.tile([128, 1], F32, tag=f"{tag}a", name=f"{tag}a")
            # seed: y_i = 0x5f3759df - (v_i >> 1)  (via xor/add trick)
            nc.vector.tensor_scalar(
                out=a[:, :].bitcast(I32), in0=v[:, :].bitcast(I32),
                scalar1=1, scalar2=None,
                op0=mybir.AluOpType.arith_shift_right)
            nc.vector.tensor_scalar(
                out=y[:, :].bitcast(I32), in0=a[:, :].bitcast(I32),
                scalar1=-1, scalar2=None,
                op0=mybir.AluOpType.bitwise_xor)
            nc.vector.tensor_scalar(
                out=y[:, :].bitcast(I32), in0=y[:, :].bitcast(I32),
                scalar1=0x5f3759e0, scalar2=None, op0=ADD)
            for _ in range(2):
                nc.vector.tensor_tensor(out=a[:, :], in0=v[:, :], in1=y[:, :],
                                        op=MUL)
                nc.vector.tensor_tensor(out=a[:, :], in0=a[:, :], in1=y[:, :],
                                        op=MUL)
                nc.vector.tensor_scalar(
                    out=a[:, :], in0=a[:, :], scalar1=-0.5, scalar2=1.5,
                    op0=MUL, op1=ADD)
                nc.vector.tensor_tensor(out=y[:, :], in0=y[:, :], in1=a[:, :],
                                        op=MUL)
            return y

        def ffn_prep(ha):
            x2t, xn2T, xn2s = [], [], []
            for jj in range(2):
                c2 = ha * 2 + jj
                rsx = sb.tile([128, D], BF16, tag="rsx", bufs=1, name=f"rsx{c2}")
                nc.sync.dma_start(out=rsx[:, :],
                                  in_=rs_out[c2 * 128:(c2 + 1) * 128, :])
                xo = sb.tile([128, D], BF16, tag="xo", bufs=1, name=f"xo{c2}")
                nc.sync.dma_start(out=xo[:, :],
                                  in_=x_own[c2 * 128:(c2 + 1) * 128, :])
                x2 = sb.tile([128, D], BF16, tag=f"x2_{jj}", bufs=2,
                             name=f"x2_{c2}")
                nc.vector.tensor_tensor(out=x2[:, :], in0=rsx[:, :],
                                        in1=xo[:, :], op=ADD)
                x2t.append(x2)
                # rsx is dead after the x2 add; reuse it as the square scratch
                ss2 = sb.tile([128, 1], F32, tag="ss2", name=f"ss2_{c2}")
                nc.scalar.activation(rsx[:, :], x2[:, :], SQUARE,
                                     accum_out=ss2[:, :])
                t2 = sb.tile([128, 1], F32, tag="t2", name=f"t2_{c2}")
                nc.vector.tensor_scalar(
                    out=t2[:, :], in0=ss2[:, :], scalar1=1.0 / D, scalar2=EPS,
                    op0=MUL, op1=ADD)
                r2 = dve_rsqrt(t2, tag=f"r2_{c2}")
                xn2 = sb.tile([128, D], BF16, tag=f"xn2_{jj}", bufs=1,
                              name=f"xn2_{c2}")
                nc.vector.tensor_scalar_mul(xn2[:, :], x2[:, :], r2[:, :])
                xn2s.append(xn2)
            for kk in range(8):
                xt2 = sb.tile([128, 256], gT.dtype, tag=f"xn2T{kk}",
                              bufs=2, name=f"xn2T{kk}_{ha}")
                xn2T.append(xt2)

            def mk_tp(jj, kk):
                def f():
                    xp = psum.tile([128, 128], BF16, tag="proj", bufs=3)
                    nc.tensor.transpose(xp[:, :],
                                        xn2s[jj][:, kk * 128:(kk + 1) * 128],
                                        ident_bf[:, :])
                    nc.vector.tensor_copy(xn2T[kk][:, jj * 128:(jj + 1) * 128],
                                          xp[:, :])
                return f

            tps = [mk_tp(jj, kk) for jj in range(2) for kk in range(8)]
            return x2t, xn2T, tps

        def ffn_mats(ha, x2t, xn2T, fillers=()):
            fillers = list(fillers)
            h_sb = []
            for m in range(16):
                if m >= 8 and fillers:
                    fillers.pop(0)()
                    if fillers:
                        fillers.pop(0)()
                gp = psum.tile([128, 256], F32, tag="sc", bufs=3)
                up = psum.tile([128, 256], F32, tag="sc", bufs=3)
                for kk in range(8):
                    nc.tensor.matmul(gp[:, :],
                                     g_sb[:, kk * DI + m * 128:kk * DI + (m + 1) * 128],
                                     xn2T[kk][:, :],
                                     start=(kk == 0), stop=(kk == 7))
                for kk in range(8):
                    nc.tensor.matmul(up[:, :],
                                     u_sb[:, kk * DI + m * 128:kk * DI + (m + 1) * 128],
                                     xn2T[kk][:, :],
                                     start=(kk == 0), stop=(kk == 7))
                sg = sb.tile([128, 256], BF16, tag="sg")
                nc.scalar.activation(sg[:, :], gp[:, :], SILU)
                hm = sb.tile([128, 256], dT.dtype, tag=f"h{m}", bufs=1,
                             name=f"h{m}_{ha}")
                nc.vector.tensor_tensor(out=hm[:, :], in0=sg[:, :],
                                        in1=up[:, :], op=MUL)
                h_sb.append(hm)

            while fillers:
                fillers.pop(0)()
            dp = [psum.tile([128, 512], F32,
                            tag="proj" if nn == 0 else "sc", bufs=3,
                            name=f"dp{jj}_{nn}")
                  for nn in range(2) for jj in range(2)]
            for m in range(16):
                for n in range(2):
                    for jj in range(2):
                        nc.tensor.matmul(
                            dp[n * 2 + jj][:, :],
                            h_sb[m][:, jj * 128:(jj + 1) * 128],
                            d_sb[:, m * D + n * 512:m * D + (n + 1) * 512],
                            start=(m == 0), stop=(m == 15))
            for jj in range(2):
                c2 = ha * 2 + jj
                osb = sb.tile([128, D], F32, tag="fout", bufs=1)
                for n in range(2):
                    nc.vector.tensor_tensor(out=osb[:, n * 512:(n + 1) * 512],
                                            in0=dp[n * 2 + jj][:, :],
                                            in1=x2t[jj][:, n * 512:(n + 1) * 512],
                                            op=ADD)
                nc.sync.dma_start(out=out[c2 * 128:(c2 + 1) * 128, :],
                                  in_=osb[:, :])


        st = stats(0)
        steps0, state0 = qkv_steps(0, st)
        for f in steps0:
            f()
        q_cur = state0["q_sb"]
        st_next = stats(1)
        state_next = None
        ffn0 = None
        for i in range(NCHUNK):
            y2_sb = sb.tile([128, 512], BF16, tag="y2_sb", name=f"y2_{i}")
            if i + 1 < NCHUNK:
                fillers, state_next = qkv_steps(i + 1, st_next)
            else:
                # last chunk: feed the g-loop + epilogue with the first FFN
                # half's transposes instead of qkv work
                x2t0, xn2T0, tps0 = ffn0
                fillers = list(tps0)
            attn_both(i, q_cur, y2_sb, fillers, reserve=16 if fillers else 0)
            if i == 1:
                for kk in range(8):
                    nc.sync.dma_start(out=g_sb[:, kk * DI:(kk + 1) * DI],
                                      in_=gT[kk * 128:(kk + 1) * 128, :])
                    nc.sync.dma_start(out=u_sb[:, kk * DI:(kk + 1) * DI],
                                      in_=uT[kk * 128:(kk + 1) * 128, :])
            if i in (2, 3):
                for m in range((i - 2) * 8, (i - 1) * 8):
                    nc.sync.dma_start(out=d_sb[:, m * D:(m + 1) * D],
                                      in_=dT[m * 128:(m + 1) * 128, :])

            if i + 2 < NCHUNK:
                st_next = stats(i + 2)
            o_proj(i, y2_sb)
            if i + 1 < NCHUNK:
                q_cur = state_next["q_sb"]
            if i == 5:
                ffn0 = ffn_prep(0)
            if i % 2 == 1:
                c = i // 2
                if no_collective:
                    nc.sync.dma_start(
                        out=rs_out[c * 128:(c + 1) * 128, :],
                        in_=rs_in[c * 1024:c * 1024 + 128, :])
                else:
                    nc.gpsimd.collective_compute(
                        "ReduceScatter", mybir.AluOpType.add,
                        ins=[rs_in[c * 1024:(c + 1) * 1024, :]],
                        outs=[rs_out[c * 128:(c + 1) * 128, :]],
                        replica_groups=[list(range(N_CORES))],
                    )

        x2t1, xn2T1, tps1 = ffn_prep(1)
        ffn_mats(0, x2t0, xn2T0, tps1)
        ffn_mats(1, x2t1, xn2T1)

    ctx.close()


# ===================== host-side sharding =====================

def make_in_maps(x, ln1_w, ln2_w, qkv_w, o_w, gate_w, up_w, down_w,
                 ffn_np_dtype=None):
    import ml_dtypes
    if ffn_np_dtype is None:
        ffn_np_dtype = ml_dtypes.bfloat16
    x = np.asarray(x, np.float32)
    xf = np.ascontiguousarray(x.reshape(NTOK, D))
    xT = np.ascontiguousarray(xf.T).astype(ml_dtypes.bfloat16)

    qkv_eff = np.asarray(qkv_w, np.float32) * np.asarray(ln1_w, np.float32)[None, :]
    g_eff = np.asarray(gate_w, np.float32) * np.asarray(ln2_w, np.float32)[None, :]
    u_eff = np.asarray(up_w, np.float32) * np.asarray(ln2_w, np.float32)[None, :]
    o_w = np.asarray(o_w, np.float32)
    down_w = np.asarray(down_w, np.float32)

    gT = np.ascontiguousarray(g_eff.T).astype(ffn_np_dtype)
    uT = np.ascontiguousarray(u_eff.T).astype(ffn_np_dtype)
    dT = np.ascontiguousarray(down_w.T).astype(ffn_np_dtype)

    scale = 1.0 / np.sqrt(HD)
    in_maps = []
    for r in range(N_CORES):
        hsl = slice(r * HPC * HD, (r + 1) * HPC * HD)  # rows for this core's heads
        qr = qkv_eff[hsl, :] * scale          # [128, D] pre-scaled q
        kr = qkv_eff[D + r * 128:D + (r + 1) * 128, :]
        vr = qkv_eff[2 * D + r * 128:2 * D + (r + 1) * 128, :]
        qkvT_r = np.ascontiguousarray(
            np.concatenate([qr, kr, vr], axis=0).T).astype(ml_dtypes.bfloat16)
        o_wT_r = np.ascontiguousarray(o_w[:, hsl].T).astype(ml_dtypes.bfloat16)
        xo = np.ascontiguousarray(
            xf.reshape(NBLK, 128, D)[r::N_CORES].reshape(512, D)).astype(
                ml_dtypes.bfloat16)
        in_maps.append({
            "xT": xT, "x_own": xo, "qkvT": qkvT_r, "o_wT": o_wT_r,
            "gT": gT, "uT": uT, "dT": dT,
        })
    return in_maps


def assemble_out(results):
    outf = np.empty((NTOK, D), np.float32)
    for r in range(N_CORES):
        outf.reshape(NBLK, 128, D)[r::N_CORES] = \
            results[r]["out"].reshape(4, 128, D)
    return outf.reshape(B, T, D)


# ===================== entry point =====================

_NC_CACHE = {}


def _get_nc():
    if "nc" not in _NC_CACHE:
        _NC_CACHE["nc"] = build_nc()
    return _NC_CACHE["nc"]


def kernel(x, ln1_w, ln2_w, qkv_w, o_w, gate_w, up_w, down_w):
    from concourse.bass_utils import run_bass_kernel_spmd

    nc = _get_nc()
    in_maps = make_in_maps(x, ln1_w, ln2_w, qkv_w, o_w, gate_w, up_w, down_w)
    res = run_bass_kernel_spmd(nc, in_maps, core_ids=list(range(N_CORES)))
    return assemble_out(res.results)
